# revision 1
# baseline (speedup 1.0000x reference)
"""Trainium2 Bass kernel for the 3-layer GAT denoising model
(nn_Denoising_Model_24764781429262): N=50000 nodes, E=800000 edges, 8 heads.

Strategy (8 NeuronCores):
- Host: add self-loops, assign each node to half A/B (balanced by degree),
  sort each half by (in-degree-from-A, in-degree-from-B), and pack nodes into
  128-node destination tiles with near-uniform padded ELL width. Tiles are
  dealt round-robin to the 8 cores; each core owns 49 contiguous table tiles.
- Per layer: each core computes its shard of a per-node table
  [H(256) | alpha_src(8) | alpha_dst(8)] with PE matmuls (fused into the
  previous layer's epilogue), AllGather's the table, then runs the edge phase:
  dma_gather of padded per-slot source rows (A/B half-tables keep int16
  indices in range), w = exp(leakyrelu(alpha_s + alpha_d)), weighted
  sum over slots on DVE -> num/denom in SBUF, divide + temb + bias + ELU.
- Final MLP is node-sharded; host inverse-permutes the output.

kernel(**inputs) takes the full unsharded inputs and returns the full
[50000, 8] float32 output.
"""

import math
import os
import numpy as np

os.environ.setdefault("NEURON_RT_RESET_CORES", "1")

import concourse.bacc as bacc
import concourse.mybir as mybir
import concourse.tile as tile
from concourse.masks import make_identity

N_CORES = 8
C = 320
HC = 256
NH = 8
F32 = mybir.dt.float32
I16 = mybir.dt.int16
AF = mybir.ActivationFunctionType
OP = mybir.AluOpType


# ----------------------------------------------------------------------------
# host preprocessing
# ----------------------------------------------------------------------------
def preprocess(adj, n, NT):
    L = 128 * NT
    PB = L + 8
    NSLOT = N_CORES * L
    HALF = 4 * PB
    E = adj.shape[1]
    src = np.concatenate([adj[0], np.arange(n)]).astype(np.int64)
    dst = np.concatenate([adj[1], np.arange(n)]).astype(np.int64)

    deg = np.bincount(dst, minlength=n)
    order_tot = np.argsort(deg, kind="stable")
    half_bit = np.zeros(n, dtype=bool)
    half_bit[order_tot[1::2]] = True
    src_is_b = half_bit[src]
    degA = np.bincount(dst[~src_is_b], minlength=n)
    degB = np.bincount(dst[src_is_b], minlength=n)

    A_nodes = np.flatnonzero(~half_bit)
    B_nodes = np.flatnonzero(half_bit)
    A_sorted = A_nodes[np.lexsort((degB[A_nodes], degA[A_nodes]))]
    B_sorted = B_nodes[np.lexsort((degB[B_nodes], degA[B_nodes]))]
    HS = NSLOT // 2
    assert len(A_sorted) <= HS and len(B_sorted) <= HS
    A_list = np.concatenate([np.full(HS - len(A_sorted), -1, np.int64), A_sorted])
    B_list = np.concatenate([np.full(HS - len(B_sorted), -1, np.int64), B_sorted])

    slots = np.full(NSLOT, -1, dtype=np.int64)
    r = np.arange(NSLOT)
    t = r // 128
    k = t % N_CORES
    i = t // N_CORES
    p = r % 128
    jA = i * 4 + k
    jB = i * 4 + (k - 4)
    selA = k < 4
    slots[selA] = A_list[jA[selA] * 128 + p[selA]]
    slots[~selA] = B_list[jB[~selA] * 128 + p[~selA]]
    physrow = k * PB + i * 128 + p
    node2phys = np.full(n, -1, dtype=np.int64)
    real = slots >= 0
    node2phys[slots[real]] = physrow[real]
    assert (node2phys >= 0).all()
    assert (node2phys[A_nodes] < HALF).all()
    assert (node2phys[B_nodes] >= HALF).all()

    dphys = node2phys[dst]
    dk = dphys // PB
    dloc = dphys % PB
    di = dloc // 128
    dp = dloc % 128

    a_cnt = np.zeros(n, np.int64)
    b_cnt = np.zeros(n, np.int64)
    np.add.at(a_cnt, dst[~src_is_b], 1)
    np.add.at(b_cnt, dst[src_is_b], 1)

    DA = np.zeros((N_CORES, NT), np.int64)
    DB = np.zeros((N_CORES, NT), np.int64)
    node_k = node2phys // PB
    node_i = (node2phys % PB) // 128
    np.maximum.at(DA, (node_k, node_i), a_cnt)
    np.maximum.at(DB, (node_k, node_i), b_cnt)
    DAi = np.maximum(DA.max(axis=0), 1)
    DBi = np.maximum(DB.max(axis=0), 1)

    # idx blocks per (core, tile, half); dummy local idx = L
    coreA = [[np.full(128 * DAi[ii], L, np.int32) for ii in range(NT)]
             for _ in range(N_CORES)]
    coreB = [[np.full(128 * DBi[ii], L, np.int32) for ii in range(NT)]
             for _ in range(N_CORES)]

    # per-(dst, half) cumulative rank
    es = np.lexsort((src, dst))
    ds_, isb_ = dst[es], src_is_b[es]
    dk_, di_, dp_ = dk[es], di[es], dp[es]
    sphys_ = node2phys[src[es]]
    keys = ds_ * 2 + isb_.astype(np.int64)
    sort2 = np.argsort(keys, kind="stable")
    ks = keys[sort2]
    starts = np.r_[0, np.flatnonzero(np.diff(ks)) + 1]
    cum = np.arange(len(ks))
    seg_start = np.repeat(cum[starts], np.diff(np.r_[starts, len(ks)]))
    rank = cum - seg_start
    jcol = np.empty(len(ks), np.int64)
    jcol[sort2] = rank
    # vectorized scatter into the per-(core,tile) blocks
    flatA_off = np.zeros((N_CORES, NT), np.int64)
    flatB_off = np.zeros((N_CORES, NT), np.int64)
    sizesA = 128 * DAi
    sizesB = 128 * DBi
    offA = np.concatenate([[0], np.cumsum(sizesA)[:-1]])
    offB = np.concatenate([[0], np.cumsum(sizesB)[:-1]])
    bigA = [np.concatenate(coreA[kk]) for kk in range(N_CORES)]
    bigB = [np.concatenate(coreB[kk]) for kk in range(N_CORES)]
    selB = isb_
    posA = offA[di_[~selB]] + jcol[~selB] * 128 + dp_[~selB]
    posB = offB[di_[selB]] + jcol[selB] * 128 + dp_[selB]
    for kk in range(N_CORES):
        mA = (~selB) & (dk_ == kk)
        bigA[kk][offA[di_[mA]] + jcol[mA] * 128 + dp_[mA]] = sphys_[mA]
        mB = selB & (dk_ == kk)
        bigB[kk][offB[di_[mB]] + jcol[mB] * 128 + dp_[mB]] = sphys_[mB] - HALF
    for kk in range(N_CORES):
        for ii in range(NT):
            coreA[kk][ii] = bigA[kk][offA[ii]:offA[ii] + sizesA[ii]]
            coreB[kk][ii] = bigB[kk][offB[ii]:offB[ii] + sizesB[ii]]

    return dict(slots=slots, node2phys=node2phys, DAi=DAi, DBi=DBi,
                coreA=coreA, coreB=coreB, n=n, NT=NT, L=L, PB=PB,
                NSLOT=NSLOT, HALF=HALF)


def build_chunks(prep, cmax):
    """Chunk plan shared by all cores: list of (tile_i, half, col0, ncols)."""
    chunks = []
    for ii in range(prep["NT"]):
        for half, D in (("A", prep["DAi"][ii]), ("B", prep["DBi"][ii])):
            c0 = 0
            while c0 < D:
                cc = min(cmax, D - c0)
                chunks.append((ii, half, c0, int(cc)))
                c0 += cc
    return chunks


def wrap_idx(block_i32):
    num = block_i32.shape[0]
    assert num % 16 == 0
    g = block_i32.reshape(num // 16, 16).T.astype(np.int16)
    return np.tile(g, (8, 1))  # [128, num/16]


def host_inputs(inputs, prep, chunks):
    """Build per-core input maps (numpy) for the bass program."""
    n, NT, L, PB = prep["n"], prep["NT"], prep["L"], prep["PB"]
    x = np.asarray(inputs["x"], np.float32)
    qY = np.asarray(inputs["q_Y_sample"], np.float32)
    NF = x.shape[1]
    F0 = NF + qY.shape[1]

    slots = prep["slots"]
    r_real = np.flatnonzero(slots >= 0)
    nodes = slots[r_real]

    # slot-order full arrays
    NS = prep["NSLOT"]
    h0 = np.zeros((NS, F0), np.float32)
    qYs = np.zeros((NS, NH), np.float32)
    h0[r_real, :NF] = x[nodes]
    h0[r_real, NF:] = qY[nodes]
    qYs[r_real] = qY[nodes]

    # per-core shard slices in slot space: core k's tile i = slot-tile t=i*8+k
    # slot index of (k, i, p) = (i*8+k)*128 + p
    def shard_rows(k):
        idx = np.empty(L, np.int64)
        for i in range(NT):
            idx[i * 128:(i + 1) * 128] = (i * N_CORES + k) * 128 + np.arange(128)
        return idx

    # weights
    W = [np.asarray(inputs[f"W{i}"], np.float32) for i in range(3)]
    att_src = np.asarray(inputs["att_src"], np.float32)
    att_dst = np.asarray(inputs["att_dst"], np.float32)
    bias = np.asarray(inputs["bias"], np.float32)
    Whats = []
    for l in range(3):
        As = np.zeros((HC, NH), np.float32)
        Ad = np.zeros((HC, NH), np.float32)
        for hh in range(NH):
            As[hh * 32:(hh + 1) * 32, hh] = att_src[l, hh]
            Ad[hh * 32:(hh + 1) * 32, hh] = att_dst[l, hh]
        Wh = np.zeros((W[l].shape[0], C), np.float32)
        Wh[:, :HC] = W[l]
        Wh[:, HC:HC + NH] = W[l] @ As
        Wh[:, HC + NH:HC + 2 * NH] = W[l] @ Ad
        Whats.append(Wh)
    # pad What0 to 136 rows already is; What1/2 264 rows.

    half = 64
    freqs4 = np.exp(np.arange(half, dtype=np.float32)
                    * (-math.log(10000.0) / (half - 1))).astype(np.float32)
    b_rep = np.stack([np.tile(bias[l][None, :], (128, 1)) for l in range(3)])

    fin_w1 = np.asarray(inputs["fin_w1"], np.float32)
    fin_b1 = np.asarray(inputs["fin_b1"], np.float32)
    fin_w2 = np.asarray(inputs["fin_w2"], np.float32)
    fin_b2 = np.asarray(inputs["fin_b2"], np.float32)

    dummy = np.zeros((8, C), np.float32)
    dummy[:, HC:HC + NH] = -1e4

    common = {
        "What0": Whats[0], "What1": Whats[1], "What2": Whats[2],
        "b_rep": b_rep.astype(np.float32),
        "fin_w1": fin_w1, "fin_b1rep": np.tile(fin_b1[None, :], (128, 1)).astype(np.float32),
        "fin_w2": fin_w2, "fin_b2rep": np.tile(fin_b2[None, :], (128, 1)).astype(np.float32),
        "tmlp_w1": np.asarray(inputs["tmlp_w1"], np.float32),
        "tmlp_b1col": np.asarray(inputs["tmlp_b1"], np.float32).reshape(128, 1),
        "tmlp_w2": np.asarray(inputs["tmlp_w2"], np.float32),
        "tmlp_b2col": np.asarray(inputs["tmlp_b2"], np.float32).reshape(256, 1)[:, :],
        "freqs4": freqs4.reshape(half, 1),
        "t_in": np.asarray(inputs["t"], np.float32).reshape(1, 1),
        "dummy_in": dummy,
    }
    # tmlp_b2col is [256,1]; split into [128,2] column pair for psum adds
    b2c = common.pop("tmlp_b2col")
    common["tmlp_b2cols"] = np.concatenate([b2c[:128], b2c[128:]], axis=1)  # [128,2]

    in_maps = []
    for k in range(N_CORES):
        rows = shard_rows(k)
        idx_blocks = []
        for (ii, hf, c0, cc) in chunks:
            blk = (prep["coreA"][k][ii] if hf == "A" else prep["coreB"][k][ii])
            sub = blk[c0 * 128:(c0 + cc) * 128]
            idx_blocks.append(wrap_idx(sub))
        idx_all = np.concatenate(idx_blocks, axis=1)  # [128, total/16]
        m = dict(common)
        m["h0T_shard"] = np.ascontiguousarray(h0[rows].T)          # [F0, L]
        m["qY_shard"] = np.ascontiguousarray(qYs[rows])            # [L, 8]
        m["idx_all"] = np.ascontiguousarray(idx_all)
        in_maps.append(m)
    return in_maps


# ----------------------------------------------------------------------------
# bass program
# ----------------------------------------------------------------------------
def build_program(prep, chunks, cmax, F0=136):
    NT, L, PB, HALF = prep["NT"], prep["L"], prep["PB"], prep["HALF"]
    NROWS = N_CORES * PB
    IDXC = sum(cc * 8 for (_, _, _, cc) in chunks)

    nc = bacc.Bacc("TRN2", target_bir_lowering=False, debug=False,
                   enable_asserts=False, num_devices=N_CORES)

    # inputs
    h0T = nc.dram_tensor("h0T_shard", [F0, L], F32, kind="ExternalInput")
    qYs = nc.dram_tensor("qY_shard", [L, NH], F32, kind="ExternalInput")
    idx_all = nc.dram_tensor("idx_all", [128, IDXC], I16, kind="ExternalInput")
    What = [nc.dram_tensor(f"What{l}", [F0 if l == 0 else 264, C], F32,
                           kind="ExternalInput") for l in range(3)]
    b_rep = nc.dram_tensor("b_rep", [3, 128, HC], F32, kind="ExternalInput")
    fin_w1 = nc.dram_tensor("fin_w1", [264, 528], F32, kind="ExternalInput")
    fin_b1rep = nc.dram_tensor("fin_b1rep", [128, 528], F32, kind="ExternalInput")
    fin_w2 = nc.dram_tensor("fin_w2", [528, NH], F32, kind="ExternalInput")
    fin_b2rep = nc.dram_tensor("fin_b2rep", [128, NH], F32, kind="ExternalInput")
    tw1 = nc.dram_tensor("tmlp_w1", [128, 128], F32, kind="ExternalInput")
    tb1c = nc.dram_tensor("tmlp_b1col", [128, 1], F32, kind="ExternalInput")
    tw2 = nc.dram_tensor("tmlp_w2", [128, HC], F32, kind="ExternalInput")
    tb2c = nc.dram_tensor("tmlp_b2cols", [128, 2], F32, kind="ExternalInput")
    freqs4 = nc.dram_tensor("freqs4", [64, 1], F32, kind="ExternalInput")
    t_in = nc.dram_tensor("t_in", [1, 1], F32, kind="ExternalInput")
    dummy_in = nc.dram_tensor("dummy_in", [8, C], F32, kind="ExternalInput")

    out = nc.dram_tensor("out", [L, NH], F32, kind="ExternalOutput")

    # internals
    AGIN = [nc.dram_tensor(f"agin{l}", [PB, C], F32, kind="Internal")
            for l in range(3)]
    T = [nc.dram_tensor(f"table{l}", [NROWS, C], F32, kind="Internal",
                        addr_space="Shared") for l in range(3)]

    with tile.TileContext(nc) as tc:
        import contextlib
        with contextlib.ExitStack() as ctx:
            consts = ctx.enter_context(tc.tile_pool(name="consts", bufs=1))
            sb = ctx.enter_context(tc.tile_pool(name="sb", bufs=3))
            sb3 = ctx.enter_context(tc.tile_pool(name="sb3", bufs=3))
            ps = ctx.enter_context(tc.tile_pool(name="ps", bufs=2, space="PSUM"))
            ps1 = ctx.enter_context(tc.tile_pool(name="ps1", bufs=1, space="PSUM"))
            gp = ctx.enter_context(tc.tile_pool(name="gp", bufs=3))

            ident = consts.tile([128, 128], F32)
            make_identity(nc, ident[:])

            # ---- dummy rows into AGIN tails
            for l in range(3):
                dt_ = consts.tile([8, C], F32, tag="dummyt")
                nc.sync.dma_start(out=dt_[:], in_=dummy_in[:])
                nc.sync.dma_start(out=AGIN[l][L:PB, :], in_=dt_[:])

            # ---- temb -> tb[l] tiles [128, 256]
            tcol = consts.tile([64, 1], F32, tag="tcol")
            nc.sync.dma_start(out=tcol[0:1, :], in_=t_in[:])
            nc.gpsimd.partition_broadcast(out_ap=tcol[:], in_ap=tcol[0:1, :])
            fq = consts.tile([64, 1], F32, tag="fq")
            nc.sync.dma_start(out=fq[:], in_=freqs4[:])
            # xs = t * 4 (t/num_steps*num_steps*rescale cancels; *4 is exact)
            xs = consts.tile([64, 1], F32, tag="xs")
            nc.vector.tensor_scalar_mul(xs[:], tcol[:], 4.0)
            ang = consts.tile([64, 1], F32, tag="ang")
            nc.vector.tensor_tensor(out=ang[:], in0=xs[:], in1=fq[:], op=OP.mult)
            # range-reduce ang into [-pi, pi]: k = floor(ang/2pi + .5), Cody-Waite
            TWO_PI = 2 * math.pi
            c1 = float(np.float32(TWO_PI))
            c2 = float(np.float32(TWO_PI - c1))
            c3 = float(TWO_PI - c1 - float(np.float32(TWO_PI - c1)))
            yk = consts.tile([64, 1], F32, tag="yk")
            nc.vector.tensor_scalar_mul(yk[:], ang[:], 1.0 / TWO_PI)
            ki = consts.tile([64, 1], mybir.dt.int32, tag="ki")
            nc.vector.tensor_copy(out=ki[:], in_=yk[:])
            kk_t = consts.tile([64, 1], F32, tag="kk_t")
            nc.vector.tensor_copy(out=kk_t[:], in_=ki[:])
            red = consts.tile([64, 1], F32, tag="red")
            nc.vector.cody_waite_cascade(out=red[:], x=ang[:], k=kk_t[:],
                                         c1=c1, c2=c2, c3=c3)
            rs = consts.tile([64, 1], F32, tag="rs")
            rc = consts.tile([64, 1], F32, tag="rc")
            nc.vector.add_range_wrap(out=rs[:], in_=red[:], shift=0.0,
                                     bound=math.pi, period=TWO_PI)
            nc.vector.add_range_wrap(out=rc[:], in_=red[:], shift=math.pi / 2,
                                     bound=math.pi, period=TWO_PI)
            sc = consts.tile([128, 1], F32, tag="sc")
            sc2 = consts.tile([64, 1], F32, tag="sc2")
            nc.scalar.activation(sc[0:64, :], rs[:], AF.Sin)
            nc.scalar.activation(sc2[:], rc[:], AF.Sin)
            nc.sync.dma_start(out=sc[64:128, :], in_=sc2[:])

            def elu_(xap, tmp_pool, shape, tag):
                # in-place ELU on xap: x = relu(x) + min(exp(x)-1, 0)
                e = tmp_pool.tile(shape, F32, tag=tag + "_e")
                r = tmp_pool.tile(shape, F32, tag=tag + "_r")
                nc.scalar.activation(e[:], xap, AF.Exp)
                nc.vector.tensor_scalar(out=e[:], in0=e[:], scalar1=-1.0,
                                        scalar2=0.0, op0=OP.add, op1=OP.min)
                nc.scalar.activation(r[:], xap, AF.Relu)
                nc.vector.tensor_tensor(out=xap, in0=e[:], in1=r[:], op=OP.add)

            tw1_s = consts.tile([128, 128], F32, tag="tw1")
            nc.sync.dma_start(out=tw1_s[:], in_=tw1[:])
            tw2_s = consts.tile([128, HC], F32, tag="tw2")
            nc.sync.dma_start(out=tw2_s[:], in_=tw2[:])
            e1p = ps1.tile([128, 1], F32, tag="tembp")
            nc.tensor.matmul(out=e1p[:], lhsT=tw1_s[:], rhs=sc[:], start=True, stop=True)
            b1c = consts.tile([128, 1], F32, tag="tb1c")
            nc.sync.dma_start(out=b1c[:], in_=tb1c[:])
            e1 = consts.tile([128, 1], F32, tag="e1")
            nc.vector.tensor_tensor(out=e1[:], in0=e1p[:], in1=b1c[:], op=OP.add)
            elu_(e1[:], consts, [128, 1], "elu_temb")
            tcols_p = ps1.tile([128, 2], F32, tag="tembp")
            nc.tensor.matmul(out=tcols_p[:, 0:1], lhsT=tw2_s[:, 0:128], rhs=e1[:],
                             start=True, stop=True)
            nc.tensor.matmul(out=tcols_p[:, 1:2], lhsT=tw2_s[:, 128:256], rhs=e1[:],
                             start=True, stop=True)
            b2c = consts.tile([128, 2], F32, tag="tb2c")
            nc.sync.dma_start(out=b2c[:], in_=tb2c[:])
            tcols = consts.tile([128, 2], F32, tag="tcols")
            nc.vector.tensor_tensor(out=tcols[:], in0=tcols_p[:], in1=b2c[:], op=OP.add)
            trow_p = ps1.tile([2, 128], F32, tag="tembp")
            nc.tensor.transpose(out=trow_p[:], in_=tcols[:], identity=ident[:])
            trow2 = consts.tile([2, 128], F32, tag="trow2")
            nc.scalar.copy(out=trow2[:], in_=trow_p[:])
            trow = consts.tile([1, HC], F32, tag="trow")
            nc.sync.dma_start(out=trow[0:1, 0:128], in_=trow2[0:1, :])
            nc.sync.dma_start(out=trow[0:1, 128:256], in_=trow2[1:2, :])
            temb_rep = consts.tile([128, HC], F32, tag="temb_rep")
            nc.gpsimd.partition_broadcast(out_ap=temb_rep[:], in_ap=trow[:])
            tb = []
            for l in range(3):
                bl = consts.tile([128, HC], F32, tag=f"b_rep{l}")
                nc.sync.dma_start(out=bl[:], in_=b_rep[l])
                tbl = consts.tile([128, HC], F32, tag=f"tb{l}")
                nc.vector.tensor_tensor(out=tbl[:], in0=temb_rep[:], in1=bl[:], op=OP.add)
                tb.append(tbl)

            # ---- layer-l What chunk tiles (load all 3 layers up-front; small)
            Wchunks = []
            for l in range(3):
                F = F0 if l == 0 else 264
                cks = []
                off = 0
                while off < F:
                    kk = min(128, F - off)
                    wt = consts.tile([kk, C], F32, tag=f"W{l}_{off}")
                    nc.sync.dma_start(out=wt[:], in_=What[l][off:off + kk, :])
                    cks.append((wt, kk))
                    off += kk
                Wchunks.append(cks)
            fw1 = []
            off = 0
            while off < 264:
                kk = min(128, 264 - off)
                wt = consts.tile([kk, 528], F32, tag=f"fw1_{off}")
                nc.sync.dma_start(out=wt[:], in_=fin_w1[off:off + kk, :])
                fw1.append((wt, kk))
                off += kk
            fw2 = []
            off = 0
            while off < 528:
                kk = min(128, 528 - off)
                wt = consts.tile([kk, NH], F32, tag=f"fw2_{off}")
                nc.sync.dma_start(out=wt[:], in_=fin_w2[off:off + kk, :])
                fw2.append((wt, kk))
                off += kk
            fb1 = consts.tile([128, 528], F32, tag="fb1")
            nc.sync.dma_start(out=fb1[:], in_=fin_b1rep[:])
            fb2 = consts.tile([128, NH], F32, tag="fb2")
            nc.sync.dma_start(out=fb2[:], in_=fin_b2rep[:])

            # ---- helper: dense T-row compute from hT chunks
            def dense_tile(hT_chunks, l, i, agin):
                pT = ps.tile([128, C], F32, tag="pT")
                ncks = len(Wchunks[l])
                for ci, ((wt, kk), (ht, kk2)) in enumerate(zip(Wchunks[l], hT_chunks)):
                    assert kk == kk2, (kk, kk2)
                    nc.tensor.matmul(out=pT[:], lhsT=ht[:kk, :], rhs=wt[:],
                                     start=(ci == 0), stop=(ci == ncks - 1))
                Trow_s = sb.tile([128, C], F32, tag="Trow_s")
                nc.scalar.copy(out=Trow_s[:], in_=pT[:])
                nc.sync.dma_start(out=agin[i * 128:(i + 1) * 128, :], in_=Trow_s[:])

            # ---- layer 0 dense: from h0T input
            for i in range(NT):
                hts = []
                off = 0
                while off < F0:
                    kk = min(128, F0 - off)
                    ht = sb.tile([128, 128], F32, tag=f"h0t_{off}")
                    nc.sync.dma_start(out=ht[:kk, 0:128],
                                      in_=h0T[off:off + kk, i * 128:(i + 1) * 128])
                    hts.append((ht, kk))
                    off += kk
                dense_tile(hts, 0, i, AGIN[0])

            # ---- per layer: allgather + edge phase
            idx_off = [0]  # running column offset in idx_all

            def edge_layer(l):
                """Edge phase for layer l; produces h_{l+1} tiles and either
                fused dense into AGIN[l+1] or the final MLP into out."""
                TA = T[l][0:HALF, :]
                TB = T[l][HALF:2 * HALF, :]
                # per-tile loop
                ch_by_tile = {}
                for ch in chunks:
                    ch_by_tile.setdefault(ch[0], []).append(ch)
                idxc = 0
                for i in range(NT):
                    acc_num = sb.tile([128, HC], F32, tag="acc_num")
                    acc_den = sb.tile([128, NH], F32, tag="acc_den")
                    first_chunk = True
                    ad_t = sb.tile([128, NH], F32, tag="ad_t")
                    row0 = 0 * PB + i * 128  # own shard: rows i*128.. of OWN block
                    # own shard rows in the big table: core k's block. SPMD: the
                    # program must address rows of ITS OWN shard. But the
                    # program is identical across cores! Use partition id?
                    # -> handled via own-shard AGIN copy: alpha_dst read from
                    #    AGIN[l] (own shard block) instead of T[l].
                    nc.sync.dma_start(
                        out=ad_t[:],
                        in_=AGIN[l][i * 128:(i + 1) * 128, HC + NH:HC + 2 * NH])
                    for (ii, hf, c0, cc) in ch_by_tile[i]:
                        tbl = TA if hf == "A" else TB
                        idx_t = sb3.tile([128, cmax * 8], I16, tag="idx_t")
                        nc.sync.dma_start(out=idx_t[:, 0:cc * 8],
                                          in_=idx_all[:, idxc:idxc + cc * 8])
                        g_t = gp.tile([128, cmax, C], F32, tag="g_t")
                        nc.gpsimd.dma_gather(
                            out_ap=g_t[:, 0:cc, :], in_ap=tbl,
                            idxs_ap=idx_t[:, 0:cc * 8],
                            num_idxs=128 * cc, num_idxs_reg=128 * cc,
                            elem_size=C, single_packet=False)
                        idxc += cc * 8
                        # logits = alpha_src + alpha_dst
                        lg = sb.tile([128, cmax, NH], F32, tag="lg")
                        nc.vector.tensor_tensor(
                            out=lg[:, 0:cc, :],
                            in0=g_t[:, 0:cc, HC:HC + NH],
                            in1=ad_t[:].unsqueeze(1).broadcast_to([128, cc, NH]),
                            op=OP.add)
                        # w = exp(leakyrelu_0.2)
                        nc.vector.scalar_tensor_tensor(
                            out=lg[:, 0:cc, :], in0=lg[:, 0:cc, :], scalar=0.2,
                            in1=lg[:, 0:cc, :], op0=OP.mult, op1=OP.max)
                        w_t = sb.tile([128, cmax, NH], F32, tag="w_t")
                        nc.scalar.activation(w_t[:, 0:cc, :], lg[:, 0:cc, :], AF.Exp)
                        # denom partial
                        dtar = acc_den if first_chunk else sb.tile([128, NH], F32, tag="dpart")
                        nc.vector.tensor_reduce(
                            out=dtar[:, :, None],
                            in_=w_t[:, 0:cc, :].rearrange("p j h -> p h j"),
                            axis=mybir.AxisListType.X, op=OP.add)
                        if not first_chunk:
                            nc.vector.tensor_tensor(out=acc_den[:], in0=acc_den[:],
                                                    in1=dtar[:], op=OP.add)
                        # num partial: tmp[p, c, j] = g[p, j, c] * w[p, j, h(c)]
                        tmp = sb.tile([128, HC, cmax], F32, tag="tmp")
                        gv = g_t[:, 0:cc, 0:HC].rearrange(
                            "p j (h c) -> p h c j", h=NH)
                        wv = w_t[:, 0:cc, :].rearrange("p j h -> p h j") \
                            .unsqueeze(2).broadcast_to([128, NH, 32, cc])
                        tv = tmp[:, :, 0:cc].rearrange("p (h c) j -> p h c j", h=NH)
                        nc.vector.tensor_tensor(out=tv, in0=gv, in1=wv, op=OP.mult)
                        ntar = acc_num if first_chunk else sb.tile([128, HC], F32, tag="npart")
                        nc.vector.tensor_reduce(
                            out=ntar[:, :, None], in_=tmp[:, :, 0:cc],
                            axis=mybir.AxisListType.X, op=OP.add)
                        if not first_chunk:
                            nc.vector.tensor_tensor(out=acc_num[:], in0=acc_num[:],
                                                    in1=ntar[:], op=OP.add)
                        first_chunk = False
                    # ---- finalize tile -> h_tile
                    rcp = sb.tile([128, NH], F32, tag="rcp")
                    nc.vector.reciprocal(rcp[:], acc_den[:])
                    h_t = sb.tile([128, 264], F32, tag="h_t")
                    nc.vector.tensor_tensor(
                        out=h_t[:, 0:HC].rearrange("p (h c) -> p h c", h=NH),
                        in0=acc_num[:].rearrange("p (h c) -> p h c", h=NH),
                        in1=rcp[:].unsqueeze(2).broadcast_to([128, NH, 32]),
                        op=OP.mult)
                    nc.vector.tensor_tensor(out=h_t[:, 0:HC], in0=h_t[:, 0:HC],
                                            in1=tb[l][:], op=OP.add)
                    elu_(h_t[:, 0:HC], sb, [128, HC], "elu_h")
                    nc.sync.dma_start(out=h_t[:, HC:264],
                                      in_=qYs[i * 128:(i + 1) * 128, :])
                    # ---- transposes
                    hts = []
                    for (off, kk, tg) in ((0, 128, "hT0"), (128, 128, "hT1"),
                                          (256, 8, "hT2")):
                        pt = ps.tile([kk, 128] if kk < 128 else [128, 128], F32,
                                     tag="ptr")
                        nc.tensor.transpose(out=pt[:kk, :],
                                            in_=h_t[:, off:off + kk],
                                            identity=ident[:])
                        st = sb.tile([kk, 128] if kk < 128 else [128, 128], F32,
                                     tag=tg)
                        nc.scalar.copy(out=st[:kk, :], in_=pt[:kk, :])
                        hts.append((st, kk))
                    if l < 2:
                        dense_tile(hts, l + 1, i, AGIN[l + 1])
                    else:
                        # final MLP
                        u = sb.tile([128, 528], F32, tag="u")
                        for half_i in range(2):
                            pm = ps1.tile([128, 264], F32, tag="pmlp")
                            for ci, (ht, kk) in enumerate(hts):
                                nc.tensor.matmul(
                                    out=pm[:],
                                    lhsT=ht[:kk, :],
                                    rhs=fw1[ci][0][:kk, half_i * 264:(half_i + 1) * 264],
                                    start=(ci == 0), stop=(ci == 2))
                            nc.vector.tensor_tensor(
                                out=u[:, half_i * 264:(half_i + 1) * 264],
                                in0=pm[:],
                                in1=fb1[:, half_i * 264:(half_i + 1) * 264],
                                op=OP.add)
                        elu_(u[:], sb, [128, 528], "elu_u")
                        po = ps1.tile([128, NH], F32, tag="po")
                        uTs = []
                        for ci in range(5):
                            off = ci * 128
                            kk = min(128, 528 - off)
                            pt = ps.tile([128, 128], F32, tag="ptr")
                            nc.tensor.transpose(out=pt[:kk, :],
                                                in_=u[:, off:off + kk],
                                                identity=ident[:])
                            st = sb.tile([128, 128], F32, tag=f"uT{ci}")
                            nc.scalar.copy(out=st[:kk, :], in_=pt[:kk, :])
                            uTs.append((st, kk))
                        for ci, (st, kk) in enumerate(uTs):
                            nc.tensor.matmul(out=po[:], lhsT=st[:kk, :],
                                             rhs=fw2[ci][0][:kk, :],
                                             start=(ci == 0), stop=(ci == 4))
                        o_t = sb.tile([128, NH], F32, tag="o_t")
                        nc.vector.tensor_tensor(out=o_t[:], in0=po[:], in1=fb2[:],
                                                op=OP.add)
                        nc.sync.dma_start(out=out[i * 128:(i + 1) * 128, :],
                                          in_=o_t[:])

            for l in range(3):
                nc.gpsimd.collective_compute(
                    "AllGather", OP.bypass,
                    replica_groups=[list(range(N_CORES))],
                    ins=[AGIN[l][:]], outs=[T[l][:]])
                edge_layer(l)

    nc.compile()
    return nc


def run(inputs, NT=49, cmax=8, trace=False):
    from concourse.bass_utils import run_bass_kernel_spmd
    from concourse.bass_interp import get_hw_module
    adj = np.asarray(inputs["adj"])
    n = int(np.asarray(inputs["x"]).shape[0])
    prep = preprocess(adj, n, NT)
    chunks = build_chunks(prep, cmax)
    in_maps = host_inputs(inputs, prep, chunks)
    F0 = in_maps[0]["h0T_shard"].shape[0]
    nc = build_program(prep, chunks, cmax, F0=F0)
    nc.m = get_hw_module(nc.m)
    res = run_bass_kernel_spmd(nc, in_maps, core_ids=list(range(N_CORES)),
                               trace=trace)
    outs = [np.asarray(r["out"]) for r in res.results]
    NS = prep["NSLOT"]
    y_slots = np.zeros((NS, NH), np.float32)
    L = prep["L"]
    for k in range(N_CORES):
        for i in range(NT):
            slot_base = (i * N_CORES + k) * 128
            y_slots[slot_base:slot_base + 128] = outs[k][i * 128:(i + 1) * 128]
    slots = prep["slots"]
    r_real = np.flatnonzero(slots >= 0)
    y = np.zeros((n, NH), np.float32)
    y[slots[r_real]] = y_slots[r_real]
    return y, res


def kernel(**inputs) -> np.ndarray:
    y, _ = run(inputs, NT=49, cmax=16)
    return y



# revision 3
# speedup vs baseline: 1.8913x; 1.8913x over previous
"""Trainium2 Bass kernel v2 for the 3-layer GAT denoising model.

Changes vs v1 baseline:
- Table rows in bf16, 768B stride (384 elems): [h interleaved c*8+h (256) |
  alpha_src (8) | alpha_dst (8) | pad]. Gather traffic 768B/edge vs 1280B.
- Channel-interleaved h layout (col = c*8 + h) so the big DVE multiply has
  a packed 2-byte last dim (4x_2p eligible) with w broadcast on a middle dim.
- qY transpose computed on host (qYT_shard input); h_t is 256 wide; only two
  PE transposes per tile.
- idx_all preloaded to SBUF once (reused by all 3 layers); x0T/qYT preloaded.
- alpha_dst preloaded per layer in one strided DMA.
- Two-level (degA//3, degB) sort for tighter ELL packing.
- AllGather in bf16, 272 cols (strided into the 384-wide table).
"""

import math
import os
import numpy as np

os.environ.setdefault("NEURON_RT_RESET_CORES", "1")

import concourse.bacc as bacc
import concourse.mybir as mybir
import concourse.tile as tile
from concourse.masks import make_identity

N_CORES = 8
TW = 384          # table row width (bf16 elems) = 768B
CW = 272          # used cols: 256 h + 8 a_src + 8 a_dst
HC = 256
NH = 8
F32 = mybir.dt.float32
BF = mybir.dt.float16     # 2-byte DVE fast path; fp16 mantissa >> bf16 here
I16 = mybir.dt.int16
AF = mybir.ActivationFunctionType
OP = mybir.AluOpType


# ----------------------------------------------------------------------------
# host preprocessing
# ----------------------------------------------------------------------------
def preprocess(adj, n, NT):
    L = 128 * NT
    PB = L + 8
    NSLOT = N_CORES * L
    HALF = 4 * PB
    src = np.concatenate([adj[0], np.arange(n)]).astype(np.int64)
    dst = np.concatenate([adj[1], np.arange(n)]).astype(np.int64)

    deg = np.bincount(dst, minlength=n)
    order_tot = np.argsort(deg, kind="stable")
    half_bit = np.zeros(n, dtype=bool)
    half_bit[order_tot[1::2]] = True
    src_is_b = half_bit[src]
    degA = np.bincount(dst[~src_is_b], minlength=n)
    degB = np.bincount(dst[src_is_b], minlength=n)

    A_nodes = np.flatnonzero(~half_bit)
    B_nodes = np.flatnonzero(half_bit)
    # sort both halves by (max(degA,degB), min(degA,degB)) so each 512-node
    # group has tight maxima in both ELL width dimensions
    dmax = np.maximum(degA, degB)
    dmin = np.minimum(degA, degB)
    A_sorted = A_nodes[np.lexsort((dmin[A_nodes], dmax[A_nodes]))]
    B_sorted = B_nodes[np.lexsort((dmin[B_nodes], dmax[B_nodes]))]
    HS = NSLOT // 2
    assert len(A_sorted) <= HS and len(B_sorted) <= HS
    A_list = np.concatenate([np.full(HS - len(A_sorted), -1, np.int64), A_sorted])
    B_list = np.concatenate([np.full(HS - len(B_sorted), -1, np.int64), B_sorted])

    slots = np.full(NSLOT, -1, dtype=np.int64)
    r = np.arange(NSLOT)
    t = r // 128
    k = t % N_CORES
    i = t // N_CORES
    p = r % 128
    jA = i * 4 + k
    jB = i * 4 + (k - 4)
    selA = k < 4
    slots[selA] = A_list[jA[selA] * 128 + p[selA]]
    slots[~selA] = B_list[jB[~selA] * 128 + p[~selA]]
    physrow = k * PB + i * 128 + p
    node2phys = np.full(n, -1, dtype=np.int64)
    real = slots >= 0
    node2phys[slots[real]] = physrow[real]
    assert (node2phys >= 0).all()
    assert (node2phys[A_nodes] < HALF).all()
    assert (node2phys[B_nodes] >= HALF).all()

    dphys = node2phys[dst]
    dk = dphys // PB
    dloc = dphys % PB
    di = dloc // 128
    dp = dloc % 128

    a_cnt = np.zeros(n, np.int64)
    b_cnt = np.zeros(n, np.int64)
    np.add.at(a_cnt, dst[~src_is_b], 1)
    np.add.at(b_cnt, dst[src_is_b], 1)

    DA = np.zeros((N_CORES, NT), np.int64)
    DB = np.zeros((N_CORES, NT), np.int64)
    node_k = node2phys // PB
    node_i = (node2phys % PB) // 128
    np.maximum.at(DA, (node_k, node_i), a_cnt)
    np.maximum.at(DB, (node_k, node_i), b_cnt)
    DAi = np.maximum(DA.max(axis=0), 1)
    DBi = np.maximum(DB.max(axis=0), 1)

    # idx blocks per (core, tile, half); dummy local idx = L
    coreA = [[np.full(128 * DAi[ii], L, np.int32) for ii in range(NT)]
             for _ in range(N_CORES)]
    coreB = [[np.full(128 * DBi[ii], L, np.int32) for ii in range(NT)]
             for _ in range(N_CORES)]

    es = np.lexsort((src, dst))
    ds_, isb_ = dst[es], src_is_b[es]
    dk_, di_, dp_ = dk[es], di[es], dp[es]
    sphys_ = node2phys[src[es]]
    keys = ds_ * 2 + isb_.astype(np.int64)
    sort2 = np.argsort(keys, kind="stable")
    ks = keys[sort2]
    starts = np.r_[0, np.flatnonzero(np.diff(ks)) + 1]
    cum = np.arange(len(ks))
    seg_start = np.repeat(cum[starts], np.diff(np.r_[starts, len(ks)]))
    rank = cum - seg_start
    jcol = np.empty(len(ks), np.int64)
    jcol[sort2] = rank
    sizesA = 128 * DAi
    sizesB = 128 * DBi
    offA = np.concatenate([[0], np.cumsum(sizesA)[:-1]])
    offB = np.concatenate([[0], np.cumsum(sizesB)[:-1]])
    bigA = [np.concatenate(coreA[kk]) for kk in range(N_CORES)]
    bigB = [np.concatenate(coreB[kk]) for kk in range(N_CORES)]
    selB = isb_
    for kk in range(N_CORES):
        mA = (~selB) & (dk_ == kk)
        bigA[kk][offA[di_[mA]] + jcol[mA] * 128 + dp_[mA]] = sphys_[mA]
        mB = selB & (dk_ == kk)
        bigB[kk][offB[di_[mB]] + jcol[mB] * 128 + dp_[mB]] = sphys_[mB] - HALF
    for kk in range(N_CORES):
        for ii in range(NT):
            coreA[kk][ii] = bigA[kk][offA[ii]:offA[ii] + sizesA[ii]]
            coreB[kk][ii] = bigB[kk][offB[ii]:offB[ii] + sizesB[ii]]

    return dict(slots=slots, node2phys=node2phys, DAi=DAi, DBi=DBi,
                coreA=coreA, coreB=coreB, n=n, NT=NT, L=L, PB=PB,
                NSLOT=NSLOT, HALF=HALF)


def build_chunks(prep, cmax):
    """Groups: [(half, [(tile, cc), ...]), ...] — adjacent tiles' same-half
    chunks share one gather. Also returns flat chunk list for idx building."""
    NT = prep["NT"]
    for ii in range(NT):
        assert prep["DAi"][ii] <= cmax and prep["DBi"][ii] <= cmax
    groups = []
    DAi, DBi = prep["DAi"], prep["DBi"]
    GCAP = 32
    i = 0
    while i < NT:
        if (i + 1 < NT and DAi[i] + DAi[i + 1] <= GCAP
                and DBi[i] + DBi[i + 1] <= GCAP):
            tiles = [i, i + 1]
        else:
            tiles = [i]
        for half, D in (("A", DAi), ("B", DBi)):
            groups.append((half, [(t, int(D[t])) for t in tiles]))
        i += len(tiles)
    return groups


def wrap_idx(block_i32):
    num = block_i32.shape[0]
    assert num % 16 == 0
    g = block_i32.reshape(num // 16, 16).T.astype(np.int16)
    return np.tile(g, (8, 1))  # [128, num/16]


# interleave: new col j = c*8+h maps to old col h*32+c
_ih = np.arange(HC)
PERM_OLD_FOR_NEW = (_ih % NH) * 32 + (_ih // NH)   # newcol j <- old PERM[j]


def host_inputs(inputs, prep, chunks):
    n, NT, L, PB = prep["n"], prep["NT"], prep["L"], prep["PB"]
    x = np.asarray(inputs["x"], np.float32)
    qY = np.asarray(inputs["q_Y_sample"], np.float32)
    NF = x.shape[1]

    slots = prep["slots"]
    r_real = np.flatnonzero(slots >= 0)
    nodes = slots[r_real]

    NS = prep["NSLOT"]
    xs = np.zeros((NS, NF), np.float32)
    qYs = np.zeros((NS, NH), np.float32)
    xs[r_real] = x[nodes]
    qYs[r_real] = qY[nodes]

    def shard_rows(k):
        idx = np.empty(L, np.int64)
        for i in range(NT):
            idx[i * 128:(i + 1) * 128] = (i * N_CORES + k) * 128 + np.arange(128)
        return idx

    W = [np.asarray(inputs[f"W{i}"], np.float32) for i in range(3)]
    att_src = np.asarray(inputs["att_src"], np.float32)
    att_dst = np.asarray(inputs["att_dst"], np.float32)
    bias = np.asarray(inputs["bias"], np.float32)
    Whats = []
    for l in range(3):
        As = np.zeros((HC, NH), np.float32)
        Ad = np.zeros((HC, NH), np.float32)
        for hh in range(NH):
            As[hh * 32:(hh + 1) * 32, hh] = att_src[l, hh]
            Ad[hh * 32:(hh + 1) * 32, hh] = att_dst[l, hh]
        Wl = W[l]
        if l > 0:
            # input rows 0:256 are the (interleaved) h of the previous layer
            Wl = np.concatenate([Wl[:HC][PERM_OLD_FOR_NEW], Wl[HC:]], axis=0)
        # Wl: rows in new input order, cols in ORIGINAL h order
        Wh = np.zeros((Wl.shape[0], TW), np.float32)
        Wh[:, :HC] = Wl[:, PERM_OLD_FOR_NEW]       # interleave output h cols
        Wh[:, HC:HC + NH] = Wl @ As
        Wh[:, HC + NH:HC + 2 * NH] = Wl @ Ad
        Whats.append(Wh.astype(np.float32))

    half = 64
    freqs4 = np.exp(np.arange(half, dtype=np.float32)
                    * (-math.log(10000.0) / (half - 1))).astype(np.float32)
    b_rep = np.stack([np.tile(bias[l][PERM_OLD_FOR_NEW][None, :], (128, 1))
                      for l in range(3)])

    fin_w1 = np.asarray(inputs["fin_w1"], np.float32)
    fin_w1 = np.concatenate([fin_w1[:HC][PERM_OLD_FOR_NEW], fin_w1[HC:]], axis=0)
    fin_b1 = np.asarray(inputs["fin_b1"], np.float32)
    fin_w2 = np.asarray(inputs["fin_w2"], np.float32)
    fin_b2 = np.asarray(inputs["fin_b2"], np.float32)

    tmlp_w2 = np.asarray(inputs["tmlp_w2"], np.float32)[:, PERM_OLD_FOR_NEW]
    tmlp_b2 = np.asarray(inputs["tmlp_b2"], np.float32)[PERM_OLD_FOR_NEW]

    dummy = np.zeros((8, TW), np.float32)
    dummy[:, HC:HC + NH] = -30000.0

    common = {
        "What0": Whats[0].astype(np.float32), "What1": Whats[1], "What2": Whats[2],
        "b_rep": b_rep.astype(np.float32),
        "fin_w1": fin_w1, "fin_b1rep": np.tile(fin_b1[None, :], (128, 1)).astype(np.float32),
        "fin_w2": fin_w2, "fin_b2rep": np.tile(fin_b2[None, :], (128, 1)).astype(np.float32),
        "tmlp_w1": np.asarray(inputs["tmlp_w1"], np.float32),
        "tmlp_b1col": np.asarray(inputs["tmlp_b1"], np.float32).reshape(128, 1),
        "tmlp_w2": tmlp_w2,
        "freqs4": freqs4.reshape(half, 1),
        "t_in": np.asarray(inputs["t"], np.float32).reshape(1, 1),
        "dummy_in": dummy,
    }
    b2c = tmlp_b2.reshape(256, 1)
    common["tmlp_b2cols"] = np.concatenate([b2c[:128], b2c[128:]], axis=1)

    # bf16 conversions
    for kk in ("What0", "What1", "What2", "fin_w1", "fin_w2", "dummy_in"):
        common[kk] = ml_bf16(common[kk])

    in_maps = []
    for k in range(N_CORES):
        rows = shard_rows(k)
        idx_blocks = []
        for (hf, tlist) in chunks:
            for (ii, cc) in tlist:
                blk = (prep["coreA"][k][ii] if hf == "A" else prep["coreB"][k][ii])
                sub = blk[0:cc * 128]
                idx_blocks.append(wrap_idx(sub))
        idx_all = np.concatenate(idx_blocks, axis=1)
        m = dict(common)
        m["x0T_shard"] = ml_bf16(np.ascontiguousarray(xs[rows].T))      # [128, L]
        m["qYT_shard"] = ml_bf16(np.ascontiguousarray(qYs[rows].T))     # [8, L]
        m["idx_all"] = np.ascontiguousarray(idx_all)
        in_maps.append(m)
    return in_maps


def ml_bf16(a):
    return np.asarray(a).astype(np.float16)


# ----------------------------------------------------------------------------
# bass program
# ----------------------------------------------------------------------------
def build_program(prep, chunks, cmax, reps=1, no_ag=False, no_gather=False,
                  queues=4):
    NT, L, PB, HALF = prep["NT"], prep["L"], prep["PB"], prep["HALF"]
    NROWS = N_CORES * PB
    IDXC = sum(cc * 8 for (_, tlist) in chunks for (_, cc) in tlist)

    nc = bacc.Bacc("TRN2", target_bir_lowering=False, debug=False,
                   enable_asserts=False, num_devices=N_CORES,
                   num_swdge_queues=queues)

    x0T = nc.dram_tensor("x0T_shard", [128, L], BF, kind="ExternalInput")
    qYT = nc.dram_tensor("qYT_shard", [NH, L], BF, kind="ExternalInput")
    idx_all = nc.dram_tensor("idx_all", [128, IDXC], I16, kind="ExternalInput")
    What = [nc.dram_tensor(f"What{l}", [136 if l == 0 else 264, TW], BF,
                           kind="ExternalInput") for l in range(3)]
    b_rep = nc.dram_tensor("b_rep", [3, 128, HC], F32, kind="ExternalInput")
    fin_w1 = nc.dram_tensor("fin_w1", [264, 528], BF, kind="ExternalInput")
    fin_b1rep = nc.dram_tensor("fin_b1rep", [128, 528], F32, kind="ExternalInput")
    fin_w2 = nc.dram_tensor("fin_w2", [528, NH], BF, kind="ExternalInput")
    fin_b2rep = nc.dram_tensor("fin_b2rep", [128, NH], F32, kind="ExternalInput")
    tw1 = nc.dram_tensor("tmlp_w1", [128, 128], F32, kind="ExternalInput")
    tb1c = nc.dram_tensor("tmlp_b1col", [128, 1], F32, kind="ExternalInput")
    tw2 = nc.dram_tensor("tmlp_w2", [128, HC], F32, kind="ExternalInput")
    tb2c = nc.dram_tensor("tmlp_b2cols", [128, 2], F32, kind="ExternalInput")
    freqs4 = nc.dram_tensor("freqs4", [64, 1], F32, kind="ExternalInput")
    t_in = nc.dram_tensor("t_in", [1, 1], F32, kind="ExternalInput")
    dummy_in = nc.dram_tensor("dummy_in", [8, TW], BF, kind="ExternalInput")

    out = nc.dram_tensor("out", [L, NH], F32, kind="ExternalOutput")

    AGIN = [nc.dram_tensor(f"agin{l}", [PB, TW], BF, kind="Internal")
            for l in range(3)]
    T = [nc.dram_tensor(f"table{l}", [NROWS, TW], BF, kind="Internal",
                        addr_space="Shared") for l in range(3)]

    with tile.TileContext(nc) as tc:
        import contextlib
        with contextlib.ExitStack() as ctx:
            ctx.enter_context(nc.allow_low_precision(reason="fp16 edge path"))
            consts = ctx.enter_context(tc.tile_pool(name="consts", bufs=1))
            sb = ctx.enter_context(tc.tile_pool(name="sb", bufs=3))
            ps = ctx.enter_context(tc.tile_pool(name="ps", bufs=2, space="PSUM"))
            ps1 = ctx.enter_context(tc.tile_pool(name="ps1", bufs=1, space="PSUM"))
            gp = ctx.enter_context(tc.tile_pool(name="gp", bufs=2))

            ident = consts.tile([128, 128], F32)
            make_identity(nc, ident[:])
            identb = consts.tile([128, 128], BF, tag="identb")
            nc.vector.tensor_copy(out=identb[:], in_=ident[:])

            # dummy rows into AGIN tails
            for l in range(3):
                dt_ = consts.tile([8, TW], BF, tag="dummyt")
                nc.sync.dma_start(out=dt_[:], in_=dummy_in[:])
                nc.sync.dma_start(out=AGIN[l][L:PB, :], in_=dt_[:])

            # ---- temb (f32, as v1) -> tb[l] tiles [128, 256] f32
            tcol = consts.tile([64, 1], F32, tag="tcol")
            nc.sync.dma_start(out=tcol[0:1, :], in_=t_in[:])
            nc.gpsimd.partition_broadcast(out_ap=tcol[:], in_ap=tcol[0:1, :])
            fq = consts.tile([64, 1], F32, tag="fq")
            nc.sync.dma_start(out=fq[:], in_=freqs4[:])
            xs_ = consts.tile([64, 1], F32, tag="xs")
            nc.vector.tensor_scalar_mul(xs_[:], tcol[:], 4.0)
            ang = consts.tile([64, 1], F32, tag="ang")
            nc.vector.tensor_tensor(out=ang[:], in0=xs_[:], in1=fq[:], op=OP.mult)
            TWO_PI = 2 * math.pi
            c1 = float(np.float32(TWO_PI))
            c2 = float(np.float32(TWO_PI - c1))
            c3 = float(TWO_PI - c1 - float(np.float32(TWO_PI - c1)))
            yk = consts.tile([64, 1], F32, tag="yk")
            nc.vector.tensor_scalar_mul(yk[:], ang[:], 1.0 / TWO_PI)
            ki = consts.tile([64, 1], mybir.dt.int32, tag="ki")
            nc.vector.tensor_copy(out=ki[:], in_=yk[:])
            kk_t = consts.tile([64, 1], F32, tag="kk_t")
            nc.vector.tensor_copy(out=kk_t[:], in_=ki[:])
            red = consts.tile([64, 1], F32, tag="red")
            nc.vector.cody_waite_cascade(out=red[:], x=ang[:], k=kk_t[:],
                                         c1=c1, c2=c2, c3=c3)
            rs = consts.tile([64, 1], F32, tag="rs")
            rc = consts.tile([64, 1], F32, tag="rc")
            nc.vector.add_range_wrap(out=rs[:], in_=red[:], shift=0.0,
                                     bound=math.pi, period=TWO_PI)
            nc.vector.add_range_wrap(out=rc[:], in_=red[:], shift=math.pi / 2,
                                     bound=math.pi, period=TWO_PI)
            sc = consts.tile([128, 1], F32, tag="sc")
            sc2 = consts.tile([64, 1], F32, tag="sc2")
            nc.scalar.activation(sc[0:64, :], rs[:], AF.Sin)
            nc.scalar.activation(sc2[:], rc[:], AF.Sin)
            nc.sync.dma_start(out=sc[64:128, :], in_=sc2[:])

            def elu_(xap, tmp_pool, shape, tag, dtype=BF):
                e = tmp_pool.tile(shape, dtype, tag=tag + "_e")
                r = tmp_pool.tile(shape, dtype, tag=tag + "_r")
                nc.scalar.activation(e[:], xap, AF.Exp)
                nc.vector.tensor_scalar(out=e[:], in0=e[:], scalar1=-1.0,
                                        scalar2=0.0, op0=OP.add, op1=OP.min)
                nc.scalar.activation(r[:], xap, AF.Relu)
                nc.vector.tensor_tensor(out=xap, in0=e[:], in1=r[:], op=OP.add)

            tw1_s = consts.tile([128, 128], F32, tag="tw1")
            nc.sync.dma_start(out=tw1_s[:], in_=tw1[:])
            tw2_s = consts.tile([128, HC], F32, tag="tw2")
            nc.sync.dma_start(out=tw2_s[:], in_=tw2[:])
            e1p = ps1.tile([128, 1], F32, tag="tembp")
            nc.tensor.matmul(out=e1p[:], lhsT=tw1_s[:], rhs=sc[:], start=True, stop=True)
            b1c = consts.tile([128, 1], F32, tag="tb1c")
            nc.sync.dma_start(out=b1c[:], in_=tb1c[:])
            e1 = consts.tile([128, 1], F32, tag="e1")
            nc.vector.tensor_tensor(out=e1[:], in0=e1p[:], in1=b1c[:], op=OP.add)
            elu_(e1[:], consts, [128, 1], "elu_temb", dtype=F32)
            tcols_p = ps1.tile([128, 2], F32, tag="tembp")
            nc.tensor.matmul(out=tcols_p[:, 0:1], lhsT=tw2_s[:, 0:128], rhs=e1[:],
                             start=True, stop=True)
            nc.tensor.matmul(out=tcols_p[:, 1:2], lhsT=tw2_s[:, 128:256], rhs=e1[:],
                             start=True, stop=True)
            b2c = consts.tile([128, 2], F32, tag="tb2c")
            nc.sync.dma_start(out=b2c[:], in_=tb2c[:])
            tcols = consts.tile([128, 2], F32, tag="tcols")
            nc.vector.tensor_tensor(out=tcols[:], in0=tcols_p[:], in1=b2c[:], op=OP.add)
            trow_p = ps1.tile([2, 128], F32, tag="tembp")
            nc.tensor.transpose(out=trow_p[:], in_=tcols[:], identity=ident[:])
            trow2 = consts.tile([2, 128], F32, tag="trow2")
            nc.scalar.copy(out=trow2[:], in_=trow_p[:])
            trow = consts.tile([1, HC], F32, tag="trow")
            nc.sync.dma_start(out=trow[0:1, 0:128], in_=trow2[0:1, :])
            nc.sync.dma_start(out=trow[0:1, 128:256], in_=trow2[1:2, :])
            temb_rep = consts.tile([128, HC], F32, tag="temb_rep")
            nc.gpsimd.partition_broadcast(out_ap=temb_rep[:], in_ap=trow[:])
            tb = []
            for l in range(3):
                bl = consts.tile([128, HC], F32, tag=f"b_rep{l}")
                nc.sync.dma_start(out=bl[:], in_=b_rep[l])
                tbl = consts.tile([128, HC], F32, tag=f"tb{l}")
                nc.vector.tensor_tensor(out=tbl[:], in0=temb_rep[:], in1=bl[:], op=OP.add)
                tb.append(tbl)

            # ---- weights into SBUF (bf16)
            Wchunks = []
            for l in range(3):
                F = 136 if l == 0 else 264
                cks = []
                off = 0
                while off < F:
                    kk = min(128, F - off)
                    wt = consts.tile([kk, TW], BF, tag=f"W{l}_{off}")
                    nc.sync.dma_start(out=wt[:], in_=What[l][off:off + kk, :])
                    cks.append((wt, kk))
                    off += kk
                Wchunks.append(cks)
            fw1 = []
            off = 0
            while off < 264:
                kk = min(128, 264 - off)
                wt = consts.tile([kk, 528], BF, tag=f"fw1_{off}")
                nc.sync.dma_start(out=wt[:], in_=fin_w1[off:off + kk, :])
                fw1.append((wt, kk))
                off += kk
            fw2 = []
            off = 0
            while off < 528:
                kk = min(128, 528 - off)
                wt = consts.tile([kk, NH], BF, tag=f"fw2_{off}")
                nc.sync.dma_start(out=wt[:], in_=fin_w2[off:off + kk, :])
                fw2.append((wt, kk))
                off += kk
            fb1 = consts.tile([128, 528], F32, tag="fb1")
            nc.sync.dma_start(out=fb1[:], in_=fin_b1rep[:])
            fb2 = consts.tile([128, NH], F32, tag="fb2")
            nc.sync.dma_start(out=fb2[:], in_=fin_b2rep[:])

            # ---- preloaded per-core data
            idx_sb = consts.tile([128, IDXC], I16, tag="idx_sb")
            nc.sync.dma_start(out=idx_sb[:], in_=idx_all[:])
            x0T_sb = consts.tile([128, L], BF, tag="x0T_sb")
            nc.sync.dma_start(out=x0T_sb[:], in_=x0T[:])
            qYT_sb = consts.tile([NH, L], BF, tag="qYT_sb")
            nc.sync.dma_start(out=qYT_sb[:], in_=qYT[:])

            # ---- dense helper: lhsT chunks (each [kk,128] bf16) -> AGIN[l] tile i
            def dense_tile(hT_chunks, l, i, agin):
                pT = ps.tile([128, TW], F32, tag="pT")
                ncks = len(Wchunks[l])
                for ci, ((wt, kk), (ht_ap, kk2)) in enumerate(zip(Wchunks[l], hT_chunks)):
                    assert kk == kk2, (kk, kk2)
                    nc.tensor.matmul(out=pT[:], lhsT=ht_ap, rhs=wt[:],
                                     start=(ci == 0), stop=(ci == ncks - 1))
                Trow_s = sb.tile([128, TW], BF, tag="Trow_s")
                nc.scalar.copy(out=Trow_s[:], in_=pT[:])
                nc.sync.dma_start(out=agin[i * 128:(i + 1) * 128, :], in_=Trow_s[:])

            # ---- layer 0 dense
            def dense0():
                for i in range(NT):
                    sl = slice(i * 128, (i + 1) * 128)
                    dense_tile([(x0T_sb[:, sl], 128), (qYT_sb[:, sl], 8)],
                               0, i, AGIN[0])
            dense0()

            # idx column offset per (half, tile) subchunk, in group order
            idx_off = {}
            off = 0
            GMAX = 0
            for (hf, tlist) in chunks:
                GMAX = max(GMAX, sum(cc for (_, cc) in tlist))
                for (ii, cc) in tlist:
                    idx_off[(hf, ii)] = off
                    off += cc * 8

            def edge_layer(l):
                TA = T[l][0:HALF, :]
                TB = T[l][HALF:2 * HALF, :]
                # preload alpha_dst for own rows: [128, NT, 8]
                ad_sb = sb.tile([128, NT, NH], BF, tag="ad_sb")
                nc.sync.dma_start(
                    out=ad_sb[:],
                    in_=AGIN[l][0:L, HC + NH:HC + 2 * NH]
                        .rearrange("(i p) h -> p i h", p=128))

                qctr = [0]

                def do_group(hf, tlist):
                    """Gather + weights + in-place weighted h + per-tile tree
                    reduce. Returns (g_tile, {tile: (col_off, den_tile)})."""
                    tbl = TA if hf == "A" else TB
                    qn = qctr[0] % queues
                    qctr[0] += 1
                    ccsum = sum(cc for (_, cc) in tlist)
                    idxc = idx_off[(hf, tlist[0][0])]
                    g_t = gp.tile([128, GMAX, TW], BF, tag=f"g{hf}")
                    if no_gather:
                        # ablation: same bytes/descriptors, sequential addresses
                        base = (tlist[0][0] * 128 * 7) % (HALF - 128 * ccsum)
                        nc.sync.dma_start(
                            out=g_t[:, 0:ccsum, :],
                            in_=tbl[base:base + 128 * ccsum, :]
                                .rearrange("(j p) w -> p j w", p=128))
                    else:
                        nc.gpsimd.dma_gather(
                            out_ap=g_t[:, 0:ccsum, :], in_ap=tbl,
                            idxs_ap=idx_sb[:, idxc:idxc + ccsum * 8],
                            num_idxs=128 * ccsum, num_idxs_reg=128 * ccsum,
                            elem_size=TW, single_packet=False, queue_num=qn)
                    # logits = a_src + a_dst per tile slice
                    lg = sb.tile([128, GMAX, NH], BF, tag="lg")
                    o = 0
                    for (ii, cc) in tlist:
                        nc.vector.tensor_tensor(
                            out=lg[:, o:o + cc, :],
                            in0=g_t[:, o:o + cc, HC:HC + NH],
                            in1=ad_sb[:, ii, :].unsqueeze(1)
                                .broadcast_to([128, cc, NH]),
                            op=OP.add)
                        o += cc
                    nc.vector.scalar_tensor_tensor(
                        out=lg[:, 0:ccsum, :], in0=lg[:, 0:ccsum, :], scalar=0.2,
                        in1=lg[:, 0:ccsum, :], op0=OP.mult, op1=OP.max)
                    w_t = sb.tile([128, GMAX, NH], BF, tag="w_t")
                    nc.scalar.activation(w_t[:, 0:ccsum, :], lg[:, 0:ccsum, :],
                                         AF.Exp)
                    # in-place weighted h: g[:, j, 0:256] *= w broadcast
                    nc.vector.tensor_tensor(
                        out=g_t[:, 0:ccsum, 0:HC].rearrange(
                            "p j (c h) -> p j c h", h=NH),
                        in0=g_t[:, 0:ccsum, 0:HC].rearrange(
                            "p j (c h) -> p j c h", h=NH),
                        in1=w_t[:, 0:ccsum, :].unsqueeze(2)
                            .broadcast_to([128, ccsum, 32, NH]),
                        op=OP.mult)
                    info = {}
                    o = 0
                    for (ii, cc) in tlist:
                        # denominator for this tile
                        den = sb.tile([128, NH], F32, tag=f"den{hf}")
                        nc.vector.tensor_reduce(
                            out=den[:, :, None],
                            in_=w_t[:, o:o + cc, :].rearrange("p j h -> p h j"),
                            axis=mybir.AxisListType.X, op=OP.add)
                        # tree-reduce weighted h over j into column o
                        k = cc
                        while k > 1:
                            h2 = k // 2
                            lo = g_t[:, o:o + h2, 0:HC]
                            hi = g_t[:, o + k - h2:o + k, 0:HC]
                            nc.vector.tensor_tensor(out=lo, in0=lo, in1=hi,
                                                    op=OP.add)
                            k -= h2
                        info[ii] = (o, den)
                        o += cc
                    return g_t, info

                for gi in range(0, len(chunks), 2):
                    (hfA, tlistA) = chunks[gi]
                    (hfB, tlistB) = chunks[gi + 1]
                    assert hfA == "A" and hfB == "B"
                    gA, infoA = do_group(hfA, tlistA)
                    gB, infoB = do_group(hfB, tlistB)
                    finalize(l, gA, infoA, gB, infoB, [t for (t, _) in tlistA])

            def finalize(l, gA, infoA, gB, infoB, tiles):
                for i in tiles:
                    oA, denA = infoA[i]
                    oB, denB = infoB[i]
                    den = sb.tile([128, NH], F32, tag="den")
                    nc.vector.tensor_tensor(out=den[:], in0=denA[:], in1=denB[:],
                                            op=OP.add)
                    rcp = sb.tile([128, NH], BF, tag="rcp")
                    nc.vector.reciprocal(rcp[:], den[:])
                    h_t = sb.tile([128, HC], BF, tag="h_t")
                    nc.vector.tensor_tensor(out=h_t[:], in0=gA[:, oA, 0:HC],
                                            in1=gB[:, oB, 0:HC], op=OP.add)
                    nc.vector.tensor_tensor(
                        out=h_t[:].rearrange("p (c h) -> p c h", h=NH),
                        in0=h_t[:].rearrange("p (c h) -> p c h", h=NH),
                        in1=rcp[:].unsqueeze(1).broadcast_to([128, 32, NH]),
                        op=OP.mult)
                    nc.vector.tensor_tensor(out=h_t[:], in0=h_t[:],
                                            in1=tb[l][:], op=OP.add)
                    elu_(h_t[:], sb, [128, HC], "elu_h")
                    # transposes (2 x 128)
                    hts = []
                    for ci, off2 in enumerate((0, 128)):
                        pt = ps.tile([128, 128], BF, tag="ptr")
                        nc.tensor.transpose(out=pt[:], in_=h_t[:, off2:off2 + 128],
                                            identity=identb[:])
                        st = sb.tile([128, 128], BF, tag=f"hT{ci}")
                        nc.scalar.copy(out=st[:], in_=pt[:])
                        hts.append((st, 128))
                    qsl = qYT_sb[:, i * 128:(i + 1) * 128]
                    if l < 2:
                        dense_tile([(hts[0][0][:], 128), (hts[1][0][:], 128),
                                    (qsl, 8)], l + 1, i, AGIN[l + 1])
                    else:
                        u = sb.tile([128, 528], BF, tag="u")
                        lhs3 = [hts[0][0][:], hts[1][0][:], qsl]
                        kks = [128, 128, 8]
                        for half_i in range(2):
                            pm = ps1.tile([128, 264], F32, tag="pmlp")
                            for ci in range(3):
                                nc.tensor.matmul(
                                    out=pm[:],
                                    lhsT=lhs3[ci],
                                    rhs=fw1[ci][0][:kks[ci],
                                                   half_i * 264:(half_i + 1) * 264],
                                    start=(ci == 0), stop=(ci == 2))
                            nc.vector.tensor_tensor(
                                out=u[:, half_i * 264:(half_i + 1) * 264],
                                in0=pm[:],
                                in1=fb1[:, half_i * 264:(half_i + 1) * 264],
                                op=OP.add)
                        elu_(u[:], sb, [128, 528], "elu_u")
                        po = ps1.tile([128, NH], F32, tag="po")
                        uTs = []
                        for ci in range(5):
                            off2 = ci * 128
                            kk = min(128, 528 - off2)
                            pt = ps.tile([128, 128], BF, tag="ptr")
                            nc.tensor.transpose(out=pt[:kk, :],
                                                in_=u[:, off2:off2 + kk],
                                                identity=identb[:])
                            st = sb.tile([128, 128], BF, tag=f"uT{ci}")
                            nc.scalar.copy(out=st[:kk, :], in_=pt[:kk, :])
                            uTs.append((st, kk))
                        for ci, (st, kk) in enumerate(uTs):
                            nc.tensor.matmul(out=po[:], lhsT=st[:kk, :],
                                             rhs=fw2[ci][0][:kk, :],
                                             start=(ci == 0), stop=(ci == 4))
                        o_t = sb.tile([128, NH], F32, tag="o_t")
                        nc.vector.tensor_tensor(out=o_t[:], in0=po[:], in1=fb2[:],
                                                op=OP.add)
                        nc.sync.dma_start(out=out[i * 128:(i + 1) * 128, :],
                                          in_=o_t[:])

            for rep in range(reps):
                if rep > 0:
                    dense0()
                for l in range(3):
                    if not no_ag:
                        nc.gpsimd.collective_compute(
                            "AllGather", OP.bypass,
                            replica_groups=[list(range(N_CORES))],
                            ins=[AGIN[l][:]], outs=[T[l][:]])
                    edge_layer(l)

    nc.compile()
    return nc


def run(inputs, NT=49, cmax=26, trace=False):
    from concourse.bass_utils import run_bass_kernel_spmd
    from concourse.bass_interp import get_hw_module
    adj = np.asarray(inputs["adj"])
    n = int(np.asarray(inputs["x"]).shape[0])
    prep = preprocess(adj, n, NT)
    chunks = build_chunks(prep, cmax)
    in_maps = host_inputs(inputs, prep, chunks)
    nc = build_program(prep, chunks, cmax)
    nc.m = get_hw_module(nc.m)
    res = run_bass_kernel_spmd(nc, in_maps, core_ids=list(range(N_CORES)),
                               trace=trace)
    outs = [np.asarray(r["out"]) for r in res.results]
    NS = prep["NSLOT"]
    y_slots = np.zeros((NS, NH), np.float32)
    for k in range(N_CORES):
        for i in range(NT):
            slot_base = (i * N_CORES + k) * 128
            y_slots[slot_base:slot_base + 128] = outs[k][i * 128:(i + 1) * 128]
    slots = prep["slots"]
    r_real = np.flatnonzero(slots >= 0)
    y = np.zeros((n, NH), np.float32)
    y[slots[r_real]] = y_slots[r_real]
    return y, res


def kernel(**inputs) -> np.ndarray:
    y, _ = run(inputs)
    return y


# revision 4
# speedup vs baseline: 3.7975x; 2.0079x over previous
"""Trainium2 Bass kernel v2 for the 3-layer GAT denoising model.

Changes vs v1 baseline:
- Table rows in bf16, 768B stride (384 elems): [h interleaved c*8+h (256) |
  alpha_src (8) | alpha_dst (8) | pad]. Gather traffic 768B/edge vs 1280B.
- Channel-interleaved h layout (col = c*8 + h) so the big DVE multiply has
  a packed 2-byte last dim (4x_2p eligible) with w broadcast on a middle dim.
- qY transpose computed on host (qYT_shard input); h_t is 256 wide; only two
  PE transposes per tile.
- idx_all preloaded to SBUF once (reused by all 3 layers); x0T/qYT preloaded.
- alpha_dst preloaded per layer in one strided DMA.
- Two-level (degA//3, degB) sort for tighter ELL packing.
- AllGather in bf16, 272 cols (strided into the 384-wide table).
"""

import math
import os
import numpy as np

os.environ.setdefault("NEURON_RT_RESET_CORES", "1")

import concourse.bacc as bacc
import concourse.mybir as mybir
import concourse.tile as tile
from concourse.masks import make_identity

N_CORES = 8
TW = 384          # table row width (bf16 elems) = 768B
CW = 272          # used cols: 256 h + 8 a_src + 8 a_dst
HC = 256
NH = 8
F32 = mybir.dt.float32
BF = mybir.dt.float16     # 2-byte DVE fast path; fp16 mantissa >> bf16 here
I16 = mybir.dt.int16
AF = mybir.ActivationFunctionType
OP = mybir.AluOpType


# ----------------------------------------------------------------------------
# host preprocessing
# ----------------------------------------------------------------------------
def preprocess(adj, n, NT):
    L = 128 * NT
    PB = L + 8
    NSLOT = N_CORES * L
    HALF = 4 * PB
    src = np.concatenate([adj[0], np.arange(n)]).astype(np.int64)
    dst = np.concatenate([adj[1], np.arange(n)]).astype(np.int64)

    deg = np.bincount(dst, minlength=n)
    order_tot = np.argsort(deg, kind="stable")
    half_bit = np.zeros(n, dtype=bool)
    half_bit[order_tot[1::2]] = True
    src_is_b = half_bit[src]
    degA = np.bincount(dst[~src_is_b], minlength=n)
    degB = np.bincount(dst[src_is_b], minlength=n)

    A_nodes = np.flatnonzero(~half_bit)
    B_nodes = np.flatnonzero(half_bit)
    # sort both halves by (max(degA,degB), min(degA,degB)) so each 512-node
    # group has tight maxima in both ELL width dimensions
    dmax = np.maximum(degA, degB)
    dmin = np.minimum(degA, degB)
    A_sorted = A_nodes[np.lexsort((dmin[A_nodes], dmax[A_nodes]))]
    B_sorted = B_nodes[np.lexsort((dmin[B_nodes], dmax[B_nodes]))]
    HS = NSLOT // 2
    assert len(A_sorted) <= HS and len(B_sorted) <= HS
    A_list = np.concatenate([np.full(HS - len(A_sorted), -1, np.int64), A_sorted])
    B_list = np.concatenate([np.full(HS - len(B_sorted), -1, np.int64), B_sorted])

    slots = np.full(NSLOT, -1, dtype=np.int64)
    r = np.arange(NSLOT)
    t = r // 128
    k = t % N_CORES
    i = t // N_CORES
    p = r % 128
    jA = i * 4 + k
    jB = i * 4 + (k - 4)
    selA = k < 4
    slots[selA] = A_list[jA[selA] * 128 + p[selA]]
    slots[~selA] = B_list[jB[~selA] * 128 + p[~selA]]
    physrow = k * PB + i * 128 + p
    node2phys = np.full(n, -1, dtype=np.int64)
    real = slots >= 0
    node2phys[slots[real]] = physrow[real]
    assert (node2phys >= 0).all()
    assert (node2phys[A_nodes] < HALF).all()
    assert (node2phys[B_nodes] >= HALF).all()

    dphys = node2phys[dst]
    dk = dphys // PB
    dloc = dphys % PB
    di = dloc // 128
    dp = dloc % 128

    a_cnt = np.zeros(n, np.int64)
    b_cnt = np.zeros(n, np.int64)
    np.add.at(a_cnt, dst[~src_is_b], 1)
    np.add.at(b_cnt, dst[src_is_b], 1)

    DA = np.zeros((N_CORES, NT), np.int64)
    DB = np.zeros((N_CORES, NT), np.int64)
    node_k = node2phys // PB
    node_i = (node2phys % PB) // 128
    np.maximum.at(DA, (node_k, node_i), a_cnt)
    np.maximum.at(DB, (node_k, node_i), b_cnt)
    DAi = np.maximum(DA.max(axis=0), 1)
    DBi = np.maximum(DB.max(axis=0), 1)

    # idx blocks per (core, tile, half); dummy local idx = L
    coreA = [[np.full(128 * DAi[ii], L, np.int32) for ii in range(NT)]
             for _ in range(N_CORES)]
    coreB = [[np.full(128 * DBi[ii], L, np.int32) for ii in range(NT)]
             for _ in range(N_CORES)]

    es = np.lexsort((src, dst))
    ds_, isb_ = dst[es], src_is_b[es]
    dk_, di_, dp_ = dk[es], di[es], dp[es]
    sphys_ = node2phys[src[es]]
    keys = ds_ * 2 + isb_.astype(np.int64)
    sort2 = np.argsort(keys, kind="stable")
    ks = keys[sort2]
    starts = np.r_[0, np.flatnonzero(np.diff(ks)) + 1]
    cum = np.arange(len(ks))
    seg_start = np.repeat(cum[starts], np.diff(np.r_[starts, len(ks)]))
    rank = cum - seg_start
    jcol = np.empty(len(ks), np.int64)
    jcol[sort2] = rank
    sizesA = 128 * DAi
    sizesB = 128 * DBi
    offA = np.concatenate([[0], np.cumsum(sizesA)[:-1]])
    offB = np.concatenate([[0], np.cumsum(sizesB)[:-1]])
    bigA = [np.concatenate(coreA[kk]) for kk in range(N_CORES)]
    bigB = [np.concatenate(coreB[kk]) for kk in range(N_CORES)]
    selB = isb_
    for kk in range(N_CORES):
        mA = (~selB) & (dk_ == kk)
        bigA[kk][offA[di_[mA]] + jcol[mA] * 128 + dp_[mA]] = sphys_[mA]
        mB = selB & (dk_ == kk)
        bigB[kk][offB[di_[mB]] + jcol[mB] * 128 + dp_[mB]] = sphys_[mB] - HALF
    for kk in range(N_CORES):
        for ii in range(NT):
            coreA[kk][ii] = bigA[kk][offA[ii]:offA[ii] + sizesA[ii]]
            coreB[kk][ii] = bigB[kk][offB[ii]:offB[ii] + sizesB[ii]]

    return dict(slots=slots, node2phys=node2phys, DAi=DAi, DBi=DBi,
                coreA=coreA, coreB=coreB, n=n, NT=NT, L=L, PB=PB,
                NSLOT=NSLOT, HALF=HALF)


def build_chunks(prep, cmax):
    """Groups: [(half, [(tile, cc), ...]), ...] — adjacent tiles' same-half
    chunks share one gather. Also returns flat chunk list for idx building."""
    NT = prep["NT"]
    for ii in range(NT):
        assert prep["DAi"][ii] <= cmax and prep["DBi"][ii] <= cmax
    groups = []
    DAi, DBi = prep["DAi"], prep["DBi"]
    GCAP = 24
    i = 0
    while i < NT:
        if (i + 1 < NT and DAi[i] + DAi[i + 1] <= GCAP
                and DBi[i] + DBi[i + 1] <= GCAP):
            tiles = [i, i + 1]
        else:
            tiles = [i]
        for half, D in (("A", DAi), ("B", DBi)):
            groups.append((half, [(t, int(D[t])) for t in tiles]))
        i += len(tiles)
    return groups


def wrap_idx(block_i32):
    num = block_i32.shape[0]
    assert num % 16 == 0
    g = block_i32.reshape(num // 16, 16).T.astype(np.int16)
    return np.tile(g, (8, 1))  # [128, num/16]


# interleave: new col j = c*8+h maps to old col h*32+c
_ih = np.arange(HC)
PERM_OLD_FOR_NEW = (_ih % NH) * 32 + (_ih // NH)   # newcol j <- old PERM[j]


def host_inputs(inputs, prep, chunks):
    n, NT, L, PB = prep["n"], prep["NT"], prep["L"], prep["PB"]
    x = np.asarray(inputs["x"], np.float32)
    qY = np.asarray(inputs["q_Y_sample"], np.float32)
    NF = x.shape[1]

    slots = prep["slots"]
    r_real = np.flatnonzero(slots >= 0)
    nodes = slots[r_real]

    NS = prep["NSLOT"]
    xs = np.zeros((NS, NF), np.float32)
    qYs = np.zeros((NS, NH), np.float32)
    xs[r_real] = x[nodes]
    qYs[r_real] = qY[nodes]

    def shard_rows(k):
        idx = np.empty(L, np.int64)
        for i in range(NT):
            idx[i * 128:(i + 1) * 128] = (i * N_CORES + k) * 128 + np.arange(128)
        return idx

    W = [np.asarray(inputs[f"W{i}"], np.float32) for i in range(3)]
    att_src = np.asarray(inputs["att_src"], np.float32)
    att_dst = np.asarray(inputs["att_dst"], np.float32)
    bias = np.asarray(inputs["bias"], np.float32)
    Whats = []
    for l in range(3):
        As = np.zeros((HC, NH), np.float32)
        Ad = np.zeros((HC, NH), np.float32)
        for hh in range(NH):
            As[hh * 32:(hh + 1) * 32, hh] = att_src[l, hh]
            Ad[hh * 32:(hh + 1) * 32, hh] = att_dst[l, hh]
        Wl = W[l]
        if l > 0:
            # input rows 0:256 are the (interleaved) h of the previous layer
            Wl = np.concatenate([Wl[:HC][PERM_OLD_FOR_NEW], Wl[HC:]], axis=0)
        # Wl: rows in new input order, cols in ORIGINAL h order
        Wh = np.zeros((Wl.shape[0], TW), np.float32)
        Wh[:, :HC] = Wl[:, PERM_OLD_FOR_NEW]       # interleave output h cols
        Wh[:, HC:HC + NH] = Wl @ As
        Wh[:, HC + NH:HC + 2 * NH] = Wl @ Ad
        Whats.append(Wh.astype(np.float32))

    half = 64
    freqs4 = np.exp(np.arange(half, dtype=np.float32)
                    * (-math.log(10000.0) / (half - 1))).astype(np.float32)
    b_rep = np.stack([np.tile(bias[l][PERM_OLD_FOR_NEW][None, :], (128, 1))
                      for l in range(3)])

    fin_w1 = np.asarray(inputs["fin_w1"], np.float32)
    fin_w1 = np.concatenate([fin_w1[:HC][PERM_OLD_FOR_NEW], fin_w1[HC:]], axis=0)
    fin_b1 = np.asarray(inputs["fin_b1"], np.float32)
    fin_w2 = np.asarray(inputs["fin_w2"], np.float32)
    fin_b2 = np.asarray(inputs["fin_b2"], np.float32)

    tmlp_w2 = np.asarray(inputs["tmlp_w2"], np.float32)[:, PERM_OLD_FOR_NEW]
    tmlp_b2 = np.asarray(inputs["tmlp_b2"], np.float32)[PERM_OLD_FOR_NEW]

    dummy = np.zeros((8, TW), np.float32)
    dummy[:, HC:HC + NH] = -30000.0

    common = {
        "What0": Whats[0].astype(np.float32), "What1": Whats[1], "What2": Whats[2],
        "b_rep": b_rep.astype(np.float32),
        "fin_w1": fin_w1, "fin_b1rep": np.tile(fin_b1[None, :], (128, 1)).astype(np.float32),
        "fin_w2": fin_w2, "fin_b2rep": np.tile(fin_b2[None, :], (128, 1)).astype(np.float32),
        "tmlp_w1": np.asarray(inputs["tmlp_w1"], np.float32),
        "tmlp_b1col": np.asarray(inputs["tmlp_b1"], np.float32).reshape(128, 1),
        "tmlp_w2": tmlp_w2,
        "freqs4": freqs4.reshape(half, 1),
        "t_in": np.asarray(inputs["t"], np.float32).reshape(1, 1),
        "dummy_in": dummy,
    }
    b2c = tmlp_b2.reshape(256, 1)
    common["tmlp_b2cols"] = np.concatenate([b2c[:128], b2c[128:]], axis=1)

    # bf16 conversions
    for kk in ("What0", "What1", "What2", "fin_w1", "fin_w2", "dummy_in"):
        common[kk] = ml_bf16(common[kk])

    in_maps = []
    for k in range(N_CORES):
        rows = shard_rows(k)
        idx_blocks = []
        for (hf, tlist) in chunks:
            for (ii, cc) in tlist:
                blk = (prep["coreA"][k][ii] if hf == "A" else prep["coreB"][k][ii])
                sub = blk[0:cc * 128]
                idx_blocks.append(wrap_idx(sub))
        idx_all = np.concatenate(idx_blocks, axis=1)
        m = dict(common)
        m["x0T_shard"] = ml_bf16(np.ascontiguousarray(xs[rows].T))      # [128, L]
        m["qYT_shard"] = ml_bf16(np.ascontiguousarray(qYs[rows].T))     # [8, L]
        m["idx_all"] = np.ascontiguousarray(idx_all)
        in_maps.append(m)
    return in_maps


def ml_bf16(a):
    return np.asarray(a).astype(np.float16)


# ----------------------------------------------------------------------------
# bass program
# ----------------------------------------------------------------------------
def build_program(prep, chunks, cmax, reps=1, no_ag=False, no_gather=False,
                  queues=4):
    NT, L, PB, HALF = prep["NT"], prep["L"], prep["PB"], prep["HALF"]
    NROWS = N_CORES * PB
    IDXC = sum(cc * 8 for (_, tlist) in chunks for (_, cc) in tlist)

    nc = bacc.Bacc("TRN2", target_bir_lowering=False, debug=False,
                   enable_asserts=False, num_devices=N_CORES,
                   num_swdge_queues=queues)

    x0T = nc.dram_tensor("x0T_shard", [128, L], BF, kind="ExternalInput")
    qYT = nc.dram_tensor("qYT_shard", [NH, L], BF, kind="ExternalInput")
    idx_all = nc.dram_tensor("idx_all", [128, IDXC], I16, kind="ExternalInput")
    What = [nc.dram_tensor(f"What{l}", [136 if l == 0 else 264, TW], BF,
                           kind="ExternalInput") for l in range(3)]
    b_rep = nc.dram_tensor("b_rep", [3, 128, HC], F32, kind="ExternalInput")
    fin_w1 = nc.dram_tensor("fin_w1", [264, 528], BF, kind="ExternalInput")
    fin_b1rep = nc.dram_tensor("fin_b1rep", [128, 528], F32, kind="ExternalInput")
    fin_w2 = nc.dram_tensor("fin_w2", [528, NH], BF, kind="ExternalInput")
    fin_b2rep = nc.dram_tensor("fin_b2rep", [128, NH], F32, kind="ExternalInput")
    tw1 = nc.dram_tensor("tmlp_w1", [128, 128], F32, kind="ExternalInput")
    tb1c = nc.dram_tensor("tmlp_b1col", [128, 1], F32, kind="ExternalInput")
    tw2 = nc.dram_tensor("tmlp_w2", [128, HC], F32, kind="ExternalInput")
    tb2c = nc.dram_tensor("tmlp_b2cols", [128, 2], F32, kind="ExternalInput")
    freqs4 = nc.dram_tensor("freqs4", [64, 1], F32, kind="ExternalInput")
    t_in = nc.dram_tensor("t_in", [1, 1], F32, kind="ExternalInput")
    dummy_in = nc.dram_tensor("dummy_in", [8, TW], BF, kind="ExternalInput")

    out = nc.dram_tensor("out", [L, NH], F32, kind="ExternalOutput")

    AGIN = [nc.dram_tensor(f"agin{l}", [PB, TW], BF, kind="Internal")
            for l in range(3)]
    T = [nc.dram_tensor(f"table{l}", [NROWS, TW], BF, kind="Internal",
                        addr_space="Shared") for l in range(3)]

    with tile.TileContext(nc) as tc:
        import contextlib
        with contextlib.ExitStack() as ctx:
            ctx.enter_context(nc.allow_low_precision(reason="fp16 edge path"))
            consts = ctx.enter_context(tc.tile_pool(name="consts", bufs=1))
            sb = ctx.enter_context(tc.tile_pool(name="sb", bufs=3))
            ps = ctx.enter_context(tc.tile_pool(name="ps", bufs=2, space="PSUM"))
            ps1 = ctx.enter_context(tc.tile_pool(name="ps1", bufs=1, space="PSUM"))
            gp = ctx.enter_context(tc.tile_pool(name="gp", bufs=3))

            ident = consts.tile([128, 128], F32)
            make_identity(nc, ident[:])
            identb = consts.tile([128, 128], BF, tag="identb")
            nc.vector.tensor_copy(out=identb[:], in_=ident[:])

            # dummy rows into AGIN tails
            for l in range(3):
                dt_ = consts.tile([8, TW], BF, tag="dummyt")
                nc.sync.dma_start(out=dt_[:], in_=dummy_in[:])
                nc.sync.dma_start(out=AGIN[l][L:PB, :], in_=dt_[:])

            # ---- temb (f32, as v1) -> tb[l] tiles [128, 256] f32
            tcol = consts.tile([64, 1], F32, tag="tcol")
            nc.sync.dma_start(out=tcol[0:1, :], in_=t_in[:])
            nc.gpsimd.partition_broadcast(out_ap=tcol[:], in_ap=tcol[0:1, :])
            fq = consts.tile([64, 1], F32, tag="fq")
            nc.sync.dma_start(out=fq[:], in_=freqs4[:])
            xs_ = consts.tile([64, 1], F32, tag="xs")
            nc.vector.tensor_scalar_mul(xs_[:], tcol[:], 4.0)
            ang = consts.tile([64, 1], F32, tag="ang")
            nc.vector.tensor_tensor(out=ang[:], in0=xs_[:], in1=fq[:], op=OP.mult)
            TWO_PI = 2 * math.pi
            c1 = float(np.float32(TWO_PI))
            c2 = float(np.float32(TWO_PI - c1))
            c3 = float(TWO_PI - c1 - float(np.float32(TWO_PI - c1)))
            yk = consts.tile([64, 1], F32, tag="yk")
            nc.vector.tensor_scalar_mul(yk[:], ang[:], 1.0 / TWO_PI)
            ki = consts.tile([64, 1], mybir.dt.int32, tag="ki")
            nc.vector.tensor_copy(out=ki[:], in_=yk[:])
            kk_t = consts.tile([64, 1], F32, tag="kk_t")
            nc.vector.tensor_copy(out=kk_t[:], in_=ki[:])
            red = consts.tile([64, 1], F32, tag="red")
            nc.vector.cody_waite_cascade(out=red[:], x=ang[:], k=kk_t[:],
                                         c1=c1, c2=c2, c3=c3)
            rs = consts.tile([64, 1], F32, tag="rs")
            rc = consts.tile([64, 1], F32, tag="rc")
            nc.vector.add_range_wrap(out=rs[:], in_=red[:], shift=0.0,
                                     bound=math.pi, period=TWO_PI)
            nc.vector.add_range_wrap(out=rc[:], in_=red[:], shift=math.pi / 2,
                                     bound=math.pi, period=TWO_PI)
            sc = consts.tile([128, 1], F32, tag="sc")
            sc2 = consts.tile([64, 1], F32, tag="sc2")
            nc.scalar.activation(sc[0:64, :], rs[:], AF.Sin)
            nc.scalar.activation(sc2[:], rc[:], AF.Sin)
            nc.sync.dma_start(out=sc[64:128, :], in_=sc2[:])

            def elu_(xap, tmp_pool, shape, tag, dtype=BF):
                e = tmp_pool.tile(shape, dtype, tag=tag + "_e")
                r = tmp_pool.tile(shape, dtype, tag=tag + "_r")
                nc.scalar.activation(e[:], xap, AF.Exp)
                nc.vector.tensor_scalar(out=e[:], in0=e[:], scalar1=-1.0,
                                        scalar2=0.0, op0=OP.add, op1=OP.min)
                nc.scalar.activation(r[:], xap, AF.Relu)
                nc.vector.tensor_tensor(out=xap, in0=e[:], in1=r[:], op=OP.add)

            tw1_s = consts.tile([128, 128], F32, tag="tw1")
            nc.sync.dma_start(out=tw1_s[:], in_=tw1[:])
            tw2_s = consts.tile([128, HC], F32, tag="tw2")
            nc.sync.dma_start(out=tw2_s[:], in_=tw2[:])
            e1p = ps1.tile([128, 1], F32, tag="tembp")
            nc.tensor.matmul(out=e1p[:], lhsT=tw1_s[:], rhs=sc[:], start=True, stop=True)
            b1c = consts.tile([128, 1], F32, tag="tb1c")
            nc.sync.dma_start(out=b1c[:], in_=tb1c[:])
            e1 = consts.tile([128, 1], F32, tag="e1")
            nc.vector.tensor_tensor(out=e1[:], in0=e1p[:], in1=b1c[:], op=OP.add)
            elu_(e1[:], consts, [128, 1], "elu_temb", dtype=F32)
            tcols_p = ps1.tile([128, 2], F32, tag="tembp")
            nc.tensor.matmul(out=tcols_p[:, 0:1], lhsT=tw2_s[:, 0:128], rhs=e1[:],
                             start=True, stop=True)
            nc.tensor.matmul(out=tcols_p[:, 1:2], lhsT=tw2_s[:, 128:256], rhs=e1[:],
                             start=True, stop=True)
            b2c = consts.tile([128, 2], F32, tag="tb2c")
            nc.sync.dma_start(out=b2c[:], in_=tb2c[:])
            tcols = consts.tile([128, 2], F32, tag="tcols")
            nc.vector.tensor_tensor(out=tcols[:], in0=tcols_p[:], in1=b2c[:], op=OP.add)
            trow_p = ps1.tile([2, 128], F32, tag="tembp")
            nc.tensor.transpose(out=trow_p[:], in_=tcols[:], identity=ident[:])
            trow2 = consts.tile([2, 128], F32, tag="trow2")
            nc.scalar.copy(out=trow2[:], in_=trow_p[:])
            trow = consts.tile([1, HC], F32, tag="trow")
            nc.sync.dma_start(out=trow[0:1, 0:128], in_=trow2[0:1, :])
            nc.sync.dma_start(out=trow[0:1, 128:256], in_=trow2[1:2, :])
            temb_rep = consts.tile([128, HC], F32, tag="temb_rep")
            nc.gpsimd.partition_broadcast(out_ap=temb_rep[:], in_ap=trow[:])
            tb = []
            for l in range(3):
                bl = consts.tile([128, HC], F32, tag=f"b_rep{l}")
                nc.sync.dma_start(out=bl[:], in_=b_rep[l])
                tbl = consts.tile([128, HC], F32, tag=f"tb{l}")
                nc.vector.tensor_tensor(out=tbl[:], in0=temb_rep[:], in1=bl[:], op=OP.add)
                tb.append(tbl)

            # ---- weights into SBUF (bf16)
            Wchunks = []
            for l in range(3):
                F = 136 if l == 0 else 264
                cks = []
                off = 0
                while off < F:
                    kk = min(128, F - off)
                    wt = consts.tile([kk, TW], BF, tag=f"W{l}_{off}")
                    nc.sync.dma_start(out=wt[:], in_=What[l][off:off + kk, :])
                    cks.append((wt, kk))
                    off += kk
                Wchunks.append(cks)
            fw1 = []
            off = 0
            while off < 264:
                kk = min(128, 264 - off)
                wt = consts.tile([kk, 528], BF, tag=f"fw1_{off}")
                nc.sync.dma_start(out=wt[:], in_=fin_w1[off:off + kk, :])
                fw1.append((wt, kk))
                off += kk
            fw2 = []
            off = 0
            while off < 528:
                kk = min(128, 528 - off)
                wt = consts.tile([kk, NH], BF, tag=f"fw2_{off}")
                nc.sync.dma_start(out=wt[:], in_=fin_w2[off:off + kk, :])
                fw2.append((wt, kk))
                off += kk
            fb1 = consts.tile([128, 528], F32, tag="fb1")
            nc.sync.dma_start(out=fb1[:], in_=fin_b1rep[:])
            fb2 = consts.tile([128, NH], F32, tag="fb2")
            nc.sync.dma_start(out=fb2[:], in_=fin_b2rep[:])

            # ---- preloaded per-core data
            idx_sb = consts.tile([128, IDXC], I16, tag="idx_sb")
            nc.sync.dma_start(out=idx_sb[:], in_=idx_all[:])
            x0T_sb = consts.tile([128, L], BF, tag="x0T_sb")
            nc.sync.dma_start(out=x0T_sb[:], in_=x0T[:])
            qYT_sb = consts.tile([NH, L], BF, tag="qYT_sb")
            nc.sync.dma_start(out=qYT_sb[:], in_=qYT[:])

            # ---- dense helper: lhsT chunks (each [kk,128] bf16) -> AGIN[l] tile i
            def dense_tile(hT_chunks, l, i, agin):
                pT = ps.tile([128, TW], F32, tag="pT")
                ncks = len(Wchunks[l])
                for ci, ((wt, kk), (ht_ap, kk2)) in enumerate(zip(Wchunks[l], hT_chunks)):
                    assert kk == kk2, (kk, kk2)
                    nc.tensor.matmul(out=pT[:], lhsT=ht_ap, rhs=wt[:],
                                     start=(ci == 0), stop=(ci == ncks - 1))
                Trow_s = sb.tile([128, TW], BF, tag="Trow_s")
                nc.scalar.copy(out=Trow_s[:], in_=pT[:])
                nc.sync.dma_start(out=agin[i * 128:(i + 1) * 128, :], in_=Trow_s[:])

            # ---- layer 0 dense
            def dense0():
                for i in range(NT):
                    sl = slice(i * 128, (i + 1) * 128)
                    dense_tile([(x0T_sb[:, sl], 128), (qYT_sb[:, sl], 8)],
                               0, i, AGIN[0])
            dense0()

            # idx column offset per (half, tile) subchunk, in group order
            idx_off = {}
            off = 0
            GMAX = 0
            for (hf, tlist) in chunks:
                GMAX = max(GMAX, sum(cc for (_, cc) in tlist))
                for (ii, cc) in tlist:
                    idx_off[(hf, ii)] = off
                    off += cc * 8

            def edge_layer(l):
                TA = T[l][0:HALF, :]
                TB = T[l][HALF:2 * HALF, :]
                # preload alpha_dst for own rows: [128, NT, 8]
                ad_sb = sb.tile([128, NT, NH], BF, tag="ad_sb")
                nc.sync.dma_start(
                    out=ad_sb[:],
                    in_=AGIN[l][0:L, HC + NH:HC + 2 * NH]
                        .rearrange("(i p) h -> p i h", p=128))

                qctr = [0]

                def do_group(hf, tlist):
                    """Gather + weights + in-place weighted h + per-tile tree
                    reduce. Returns (g_tile, {tile: (col_off, den_tile)})."""
                    tbl = TA if hf == "A" else TB
                    qn = qctr[0] % queues
                    qctr[0] += 1
                    ccsum = sum(cc for (_, cc) in tlist)
                    idxc = idx_off[(hf, tlist[0][0])]
                    g_t = gp.tile([128, GMAX, TW], BF, tag=f"g{hf}")
                    if no_gather:
                        # ablation: same bytes/descriptors, sequential addresses
                        base = (tlist[0][0] * 128 * 7) % (HALF - 128 * ccsum)
                        nc.sync.dma_start(
                            out=g_t[:, 0:ccsum, :],
                            in_=tbl[base:base + 128 * ccsum, :]
                                .rearrange("(j p) w -> p j w", p=128))
                    else:
                        nc.gpsimd.dma_gather(
                            out_ap=g_t[:, 0:ccsum, :], in_ap=tbl,
                            idxs_ap=idx_sb[:, idxc:idxc + ccsum * 8],
                            num_idxs=128 * ccsum, num_idxs_reg=128 * ccsum,
                            elem_size=TW, single_packet=False, queue_num=qn)
                    # logits = a_src + a_dst per tile slice
                    lg = sb.tile([128, GMAX, NH], BF, tag="lg")
                    o = 0
                    for (ii, cc) in tlist:
                        nc.vector.tensor_tensor(
                            out=lg[:, o:o + cc, :],
                            in0=g_t[:, o:o + cc, HC:HC + NH],
                            in1=ad_sb[:, ii, :].unsqueeze(1)
                                .broadcast_to([128, cc, NH]),
                            op=OP.add)
                        o += cc
                    nc.vector.scalar_tensor_tensor(
                        out=lg[:, 0:ccsum, :], in0=lg[:, 0:ccsum, :], scalar=0.2,
                        in1=lg[:, 0:ccsum, :], op0=OP.mult, op1=OP.max)
                    w_t = sb.tile([128, GMAX, NH], BF, tag="w_t")
                    nc.scalar.activation(w_t[:, 0:ccsum, :], lg[:, 0:ccsum, :],
                                         AF.Exp)
                    # in-place weighted h: g[:, j, 0:256] *= w broadcast
                    nc.vector.tensor_tensor(
                        out=g_t[:, 0:ccsum, 0:HC].rearrange(
                            "p j (c h) -> p j c h", h=NH),
                        in0=g_t[:, 0:ccsum, 0:HC].rearrange(
                            "p j (c h) -> p j c h", h=NH),
                        in1=w_t[:, 0:ccsum, :].unsqueeze(2)
                            .broadcast_to([128, ccsum, 32, NH]),
                        op=OP.mult)
                    info = {}
                    o = 0
                    for (ii, cc) in tlist:
                        # denominator for this tile
                        den = sb.tile([128, NH], F32, tag=f"den{hf}")
                        nc.vector.tensor_reduce(
                            out=den[:, :, None],
                            in_=w_t[:, o:o + cc, :].rearrange("p j h -> p h j"),
                            axis=mybir.AxisListType.X, op=OP.add)
                        # tree-reduce weighted h over j into column o
                        k = cc
                        while k > 1:
                            h2 = k // 2
                            lo = g_t[:, o:o + h2, 0:HC]
                            hi = g_t[:, o + k - h2:o + k, 0:HC]
                            nc.vector.tensor_tensor(out=lo, in0=lo, in1=hi,
                                                    op=OP.add)
                            k -= h2
                        info[ii] = (o, den)
                        o += cc
                    return g_t, info

                for gi in range(0, len(chunks), 2):
                    (hfA, tlistA) = chunks[gi]
                    (hfB, tlistB) = chunks[gi + 1]
                    assert hfA == "A" and hfB == "B"
                    gA, infoA = do_group(hfA, tlistA)
                    gB, infoB = do_group(hfB, tlistB)
                    finalize(l, gA, infoA, gB, infoB, [t for (t, _) in tlistA])

            def finalize(l, gA, infoA, gB, infoB, tiles):
                for i in tiles:
                    oA, denA = infoA[i]
                    oB, denB = infoB[i]
                    den = sb.tile([128, NH], F32, tag="den")
                    nc.vector.tensor_tensor(out=den[:], in0=denA[:], in1=denB[:],
                                            op=OP.add)
                    rcp = sb.tile([128, NH], BF, tag="rcp")
                    nc.vector.reciprocal(rcp[:], den[:])
                    h_t = sb.tile([128, HC], BF, tag="h_t")
                    nc.vector.tensor_tensor(out=h_t[:], in0=gA[:, oA, 0:HC],
                                            in1=gB[:, oB, 0:HC], op=OP.add)
                    nc.vector.tensor_tensor(
                        out=h_t[:].rearrange("p (c h) -> p c h", h=NH),
                        in0=h_t[:].rearrange("p (c h) -> p c h", h=NH),
                        in1=rcp[:].unsqueeze(1).broadcast_to([128, 32, NH]),
                        op=OP.mult)
                    nc.vector.tensor_tensor(out=h_t[:], in0=h_t[:],
                                            in1=tb[l][:], op=OP.add)
                    elu_(h_t[:], sb, [128, HC], "elu_h")
                    # transposes (2 x 128)
                    hts = []
                    for ci, off2 in enumerate((0, 128)):
                        pt = ps.tile([128, 128], BF, tag="ptr")
                        nc.tensor.transpose(out=pt[:], in_=h_t[:, off2:off2 + 128],
                                            identity=identb[:])
                        st = sb.tile([128, 128], BF, tag=f"hT{ci}")
                        nc.scalar.copy(out=st[:], in_=pt[:])
                        hts.append((st, 128))
                    qsl = qYT_sb[:, i * 128:(i + 1) * 128]
                    if l < 2:
                        dense_tile([(hts[0][0][:], 128), (hts[1][0][:], 128),
                                    (qsl, 8)], l + 1, i, AGIN[l + 1])
                    else:
                        u = sb.tile([128, 528], BF, tag="u")
                        lhs3 = [hts[0][0][:], hts[1][0][:], qsl]
                        kks = [128, 128, 8]
                        for half_i in range(2):
                            pm = ps1.tile([128, 264], F32, tag="pmlp")
                            for ci in range(3):
                                nc.tensor.matmul(
                                    out=pm[:],
                                    lhsT=lhs3[ci],
                                    rhs=fw1[ci][0][:kks[ci],
                                                   half_i * 264:(half_i + 1) * 264],
                                    start=(ci == 0), stop=(ci == 2))
                            nc.vector.tensor_tensor(
                                out=u[:, half_i * 264:(half_i + 1) * 264],
                                in0=pm[:],
                                in1=fb1[:, half_i * 264:(half_i + 1) * 264],
                                op=OP.add)
                        elu_(u[:], sb, [128, 528], "elu_u")
                        po = ps1.tile([128, NH], F32, tag="po")
                        uTs = []
                        for ci in range(5):
                            off2 = ci * 128
                            kk = min(128, 528 - off2)
                            pt = ps.tile([128, 128], BF, tag="ptr")
                            nc.tensor.transpose(out=pt[:kk, :],
                                                in_=u[:, off2:off2 + kk],
                                                identity=identb[:])
                            st = sb.tile([128, 128], BF, tag=f"uT{ci}")
                            nc.scalar.copy(out=st[:kk, :], in_=pt[:kk, :])
                            uTs.append((st, kk))
                        for ci, (st, kk) in enumerate(uTs):
                            nc.tensor.matmul(out=po[:], lhsT=st[:kk, :],
                                             rhs=fw2[ci][0][:kk, :],
                                             start=(ci == 0), stop=(ci == 4))
                        o_t = sb.tile([128, NH], F32, tag="o_t")
                        nc.vector.tensor_tensor(out=o_t[:], in0=po[:], in1=fb2[:],
                                                op=OP.add)
                        nc.sync.dma_start(out=out[i * 128:(i + 1) * 128, :],
                                          in_=o_t[:])

            for rep in range(reps):
                if rep > 0:
                    dense0()
                for l in range(3):
                    if not no_ag:
                        nc.gpsimd.collective_compute(
                            "AllGather", OP.bypass,
                            replica_groups=[list(range(N_CORES))],
                            ins=[AGIN[l][:]], outs=[T[l][:]])
                    edge_layer(l)

    nc.compile()
    return nc


def run(inputs, NT=49, cmax=26, trace=False):
    from concourse.bass_utils import run_bass_kernel_spmd
    from concourse.bass_interp import get_hw_module
    adj = np.asarray(inputs["adj"])
    n = int(np.asarray(inputs["x"]).shape[0])
    prep = preprocess(adj, n, NT)
    chunks = build_chunks(prep, cmax)
    in_maps = host_inputs(inputs, prep, chunks)
    nc = build_program(prep, chunks, cmax)
    nc.m = get_hw_module(nc.m)
    res = run_bass_kernel_spmd(nc, in_maps, core_ids=list(range(N_CORES)),
                               trace=trace)
    outs = [np.asarray(r["out"]) for r in res.results]
    NS = prep["NSLOT"]
    y_slots = np.zeros((NS, NH), np.float32)
    for k in range(N_CORES):
        for i in range(NT):
            slot_base = (i * N_CORES + k) * 128
            y_slots[slot_base:slot_base + 128] = outs[k][i * 128:(i + 1) * 128]
    slots = prep["slots"]
    r_real = np.flatnonzero(slots >= 0)
    y = np.zeros((n, NH), np.float32)
    y[slots[r_real]] = y_slots[r_real]
    return y, res


def kernel(**inputs) -> np.ndarray:
    y, _ = run(inputs)
    return y
